# revision 25
# baseline (speedup 1.0000x reference)
"""Trainium2 Bass kernel for nn_Discriminator (decomposable attention over
gathered embeddings).

Math (reference):
    p_emb = emb[p_idx]; h_emb = emb[h_idx]                # [4096, 300]
    fp = attend(p_emb); fh = attend(h_emb)                # [4096, 512]
    G  = fh.reshape(512, 4096)      (row-major reshape)
    E  = fp @ G                                           # [4096, 4096]
    eik = E.sum(1); ekj = E.sum(0)
    beta  = (E/eik) @ h_emb;  alpha = (E/ekj).T @ p_emb   # [4096, 300]
    v1 = comp([p_emb|beta]).sum(0); v2 = comp([h_emb|alpha]).sum(0)
    y  = softmax(mlp([v1|v2]))                            # [3]

Key identities used to shard across 8 cores without collectives
(G[k, r*512+c] == fh[8k+r, c], so G's column block r is fh[r::8]):
    eik = fp @ g,          g = G.sum(1)
    E @ h_emb = fp @ T,    T = sum_r fh[r::8] @ h_emb[r*512:(r+1)*512]
    ekj[r*512+c] = (fh[r::8].T @ sfp)[c],   sfp = fp.sum(0)
    (E.T @ p_emb)[r*512:(r+1)*512] = fh[r::8].T @ S,   S = fp.T @ p_emb

Two SPMD launches on cores 0-7:
    L1: per-core attend on its p-block (rows c*512:(c+1)*512) and its strided
        h-slice (rows r::8); partial S_c, T_r; the diagonal E block
        fp_c @ fh_r (core c owns both operands).  Host sums S/T (tiny) and
        assembles G.
    L2: per-core remaining 7 E column chunks of its row block (G pack is
        rotated per core so the single NEFF stays SPMD), beta/alpha blocks,
        v1/v2 partials.
Host does only O(KB) glue plus the final 3-way MLP head on [v1|v2].

All device inputs/outputs are pre-swizzled on the host into [128, N]
partition-major flats so every DMA is one contiguous line per partition
(sequencer descriptor-generation cost was a profiled bottleneck), and input
packs are ordered so first-needed operands complete first (DMA bandwidth is
the other profiled bottleneck).
"""

import numpy as np

_P = 128
_D = 300
_H = 512
_L = 4096
_B = 512  # rows per core
_NCORES = 8
_DPAD = 384  # 300 padded up to 3*128 (row 300 carries the ones/bias trick)
_DN = _D + 2  # fp32r needs an even moving dim; col 300 = normalizer, 301 pad

_HK = _H // _P  # 4
_DK = _DPAD // _P  # 3
_NE = _NCORES - 1  # 7 off-diagonal E column chunks in L2

_cache = {}
LAST_RESULTS = []  # BassKernelResults of the most recent kernel() launches


def _swz(a, t):
    """[t*128, n] row-major -> [128, t*n] partition-major flat."""
    n = a.shape[1]
    return a.reshape(t, _P, n).transpose(1, 0, 2).reshape(_P, t * n)


def _unswz(a, t):
    """[128, t*n] partition-major flat -> [t*128, n] row-major."""
    n = a.shape[1] // t
    return a.reshape(_P, t, n).transpose(1, 0, 2).reshape(t * _P, n)


def _pad_rows(a, rows):
    out = np.zeros((rows, a.shape[1]), np.float32)
    out[: a.shape[0]] = a
    return out


class _Pack:
    """Host-side [128, N] pack builder + device-side view registry."""

    def __init__(self):
        self.specs = []  # (name, off, t, n)
        self.total = 0

    def add(self, name, t, n):
        self.specs.append((name, self.total, t, n))
        self.total += t * n

    def off(self, name):
        for nm, off, t, n in self.specs:
            if nm == name:
                return off, t, n
        raise KeyError(name)

    def view(self, tile, name):
        off, t, n = self.off(name)
        return tile[:, off : off + t * n].rearrange("p (t n) -> p t n", t=t)

    def range_of(self, names):
        offs = []
        for nm in names:
            off, t, n = self.off(nm)
            offs.append((off, off + t * n))
        lo = min(o for o, _ in offs)
        hi = max(e for _, e in offs)
        assert hi - lo == sum(e - o for o, e in offs), "group must be contiguous"
        return lo, hi

    def build(self, arrays, dtype=np.float32):
        """arrays: {name: [t*128, n] array}; returns [128, total] in dtype."""
        out = np.empty((_P, self.total), dtype)
        for nm, off, t, n in self.specs:
            a = arrays[nm]
            assert a.shape == (t * _P, n), (nm, a.shape, (t * _P, n))
            out[:, off : off + t * n] = _swz(
                np.ascontiguousarray(a).astype(dtype), t
            )
        return out


# ---- pack layouts (module-level so host and builder agree) ----
_PK1A = _Pack()  # L1 sync: attend-p operands, then S/T rhs
_PK1A.add("w1b", _DK, _H)
_PK1A.add("ptb", _DK, _B)
_PK1A.add("pblk", _HK, _D)
_PK1A.add("hblk", _HK, _D)
_PK1B = _Pack()  # L1 scalar: attend layer-2 weights + attend-h input
_PK1B.add("w2", _HK, _H)
_PK1B.add("ba2", 1, _HK)
_PK1B.add("htb", _DK, _B)

_PK2A = _Pack()  # L2 sync fp16, ahead of the G chunks: E lhsT
_PK2A.add("fpT16", _HK, _B)
_PK2B = _Pack()  # L2 scalar f32: beta/alpha operands (f32r quality path)
_PK2B.add("Tg", _HK, _DN)
_PK2B.add("Ss", _HK, _DN)
_PK2B.add("fpT", _HK, _B)
_PK2B.add("fhr", _HK, _H)
_PK2B.add("bc1", 1, _HK)
_PK2B.add("bc2", 1, _HK)
_PK2C = _Pack()  # L2 scalar fp16: comp operands
_PK2C.add("wc1p", _DK, _H)
_PK2C.add("wc1b", _DK, _H)
_PK2C.add("wc2", _HK, _H)
_PK2C.add("pT", _DK, _B)
_PK2C.add("hT", _DK, _B)


def _build_l1():
    import concourse.bacc as bacc
    import concourse.bass as bass
    import concourse.mybir as mybir
    import concourse.tile as tile
    from concourse.masks import make_identity

    F32 = mybir.dt.float32
    F32R = mybir.dt.float32r
    ts = bass.ts

    nc = bacc.Bacc("TRN2", target_bir_lowering=False, debug=False, num_devices=_NCORES)

    pk_a = nc.dram_tensor("pk_a", [_P, _PK1A.total], F32, kind="ExternalInput")
    pk_b = nc.dram_tensor("pk_b", [_P, _PK1B.total], F32, kind="ExternalInput")

    fpT_o = nc.dram_tensor("fpT", [_P, _HK * _B], F32, kind="ExternalOutput")
    fhT_o = nc.dram_tensor("fhT", [_P, _HK * _B], F32, kind="ExternalOutput")
    ST_o = nc.dram_tensor("ST", [_P, 2 * _HK * _D], F32, kind="ExternalOutput")
    ED_o = nc.dram_tensor("ED", [_P, _HK * _B], F32, kind="ExternalOutput")

    with tile.TileContext(nc) as tc:
        with (
            tc.tile_pool(name="consts", bufs=1) as cb,
            tc.tile_pool(name="one", bufs=1) as ob,
            tc.tile_pool(name="sbuf", bufs=2) as sb,
            tc.tile_pool(name="psum", bufs=2, space="PSUM") as pp,
            tc.tile_pool(name="edpsum", bufs=2, space="PSUM") as ep,
        ):
            ta = cb.tile([_P, _PK1A.total], F32R)
            lo, hi = _PK1A.range_of(["w1b", "ptb"])
            nc.sync.dma_start(ta[:, lo:hi], pk_a[:, lo:hi].bitcast(F32R))
            lo2, hi2 = _PK1A.range_of(["pblk", "hblk"])
            nc.sync.dma_start(ta[:, lo2:hi2], pk_a[:, lo2:hi2].bitcast(F32R))
            tb = cb.tile([_P, _PK1B.total], F32R)
            nc.scalar.dma_start(tb[:], pk_b[:].bitcast(F32R))

            ident = cb.tile([_P, _P], F32)
            make_identity(nc, ident[:])

            # PE pre-warm: ~4us of junk matmuls during the input-DMA wait so
            # the HAM clock gate opens (1.2 -> 2.4 GHz) before real work.
            warm_ps = pp.tile([_P, _B], F32, tag="attps")
            for _ in range(24):
                nc.tensor.matmul(
                    warm_ps[:, 0:_P], ident[:], ident[:], start=True, stop=True
                )

            w1b_t = _PK1A.view(ta, "w1b")
            ptb_t = _PK1A.view(ta, "ptb")
            pblk_t = _PK1A.view(ta, "pblk")
            hblk_t = _PK1A.view(ta, "hblk")
            w2_t = _PK1B.view(tb, "w2")
            ba2_t = _PK1B.view(tb, "ba2")[:, 0, :].bitcast(F32)
            htb_t = _PK1B.view(tb, "htb")

            def attend_T(xt):
                z1 = sb.tile([_P, _HK, _B], F32R, tag="attz1")
                for mt in range(_HK):
                    ps = pp.tile([_P, _B], F32, tag="attps")
                    for kt in range(_DK):
                        nc.tensor.matmul(
                            ps[:],
                            w1b_t[:, kt, ts(mt, _P)],
                            xt[:, kt, :],
                            start=(kt == 0),
                            stop=(kt == _DK - 1),
                        )
                    nc.scalar.activation(
                        z1[:, mt, :], ps[:], mybir.ActivationFunctionType.Relu
                    )
                fT = sb.tile([_P, _HK, _B], F32R, tag="attout")
                for mt in range(_HK):
                    ps = pp.tile([_P, _B], F32, tag="attps")
                    for kt in range(_HK):
                        nc.tensor.matmul(
                            ps[:],
                            w2_t[:, kt, ts(mt, _P)],
                            z1[:, kt, :],
                            start=(kt == 0),
                            stop=(kt == _HK - 1),
                        )
                    nc.scalar.activation(
                        fT[:, mt, :],
                        ps[:],
                        mybir.ActivationFunctionType.Relu,
                        bias=ba2_t[:, mt : mt + 1],
                    )
                return fT

            fpT = attend_T(ptb_t)
            nc.sync.dma_start(fpT_o[:].bitcast(F32R), fpT[:])
            fhT = attend_T(htb_t)
            nc.scalar.dma_start(fhT_o[:].bitcast(F32R), fhT[:])

            def transpose_16(src):
                rm = ob.tile([_P, _HK, _H], F32R, tag=f"rm{src is fhT}")
                for i in range(_HK):
                    for j in range(_HK):
                        tp = pp.tile([_P, _P], F32, tag="tps")
                        nc.tensor.transpose(
                            tp[:], src[:, i, ts(j, _P)].bitcast(F32), ident[:]
                        )
                        nc.vector.tensor_copy(rm[:, j, ts(i, _P)], tp[:].bitcast(F32R))
                return rm

            st_sb = ob.tile([_P, 2, _HK, _D], F32)
            # T_r[k, d] = sum_c fh_r[k, c] * h_blk[c, d]  (lhsT = fhT directly)
            for mt in range(_HK):
                ps = pp.tile([_P, _D], F32, tag="stps")
                for kt in range(_HK):
                    nc.tensor.matmul(
                        ps[:],
                        fhT[:, kt, ts(mt, _P)],
                        hblk_t[:, kt, :],
                        start=(kt == 0),
                        stop=(kt == _HK - 1),
                    )
                nc.vector.tensor_copy(st_sb[:, 1, mt, :], ps[:])
            nc.scalar.dma_start(ST_o[:, _HK * _D :], st_sb[:, 1, :, :])

            fh_rm = transpose_16(fhT)  # fh_r row-major: E diag chunk's rhs

            # E diagonal block: fp_c @ G[:, r*512:(r+1)*512] = fp_c @ fh_r
            ed = ob.tile([_P, _HK, _B], F32)
            for mt in range(_HK):
                ps = ep.tile([_P, _B], F32, tag="edps")
                for kt in range(_HK):
                    nc.tensor.matmul(
                        ps[:],
                        fpT[:, kt, ts(mt, _P)],
                        fh_rm[:, kt, :],
                        start=(kt == 0),
                        stop=(kt == _HK - 1),
                    )
                nc.vector.tensor_copy(ed[:, mt, :], ps[:])
            nc.scalar.dma_start(ED_o[:], ed[:])

            fp_rm = transpose_16(fpT)  # fp row-major: S's lhsT

            # S_c[k, d] = sum_i fp[i, k] * p_emb[i, d]
            for mt in range(_HK):
                ps = pp.tile([_P, _D], F32, tag="stps")
                for kt in range(_HK):
                    nc.tensor.matmul(
                        ps[:],
                        fp_rm[:, kt, ts(mt, _P)],
                        pblk_t[:, kt, :],
                        start=(kt == 0),
                        stop=(kt == _HK - 1),
                    )
                nc.vector.tensor_copy(st_sb[:, 0, mt, :], ps[:])
            nc.sync.dma_start(ST_o[:, : _HK * _D], st_sb[:, 0, :, :])

    nc.compile()
    return nc


def _build_l2():
    import concourse.bacc as bacc
    import concourse.bass as bass
    import concourse.mybir as mybir
    import concourse.tile as tile
    from concourse.masks import make_identity

    F32 = mybir.dt.float32
    F32R = mybir.dt.float32r
    F16 = mybir.dt.float16
    ts = bass.ts

    nc = bacc.Bacc("TRN2", target_bir_lowering=False, debug=False, num_devices=_NCORES)

    pk_a = nc.dram_tensor("pk_a", [_P, _PK2A.total], F16, kind="ExternalInput")
    pk_b = nc.dram_tensor("pk_b", [_P, _PK2B.total], F32, kind="ExternalInput")
    pk_c = nc.dram_tensor("pk_c", [_P, _PK2C.total], F16, kind="ExternalInput")
    # G packed chunk-major, 7 per-core-rotated chunks: [p][j][kt][512]
    G_i = nc.dram_tensor("G", [_P, _NE * _HK * _B], F16, kind="ExternalInput")

    # E packed chunk-major [p][j][mt][512]; host unswizzles + unrotates
    E_o = nc.dram_tensor("E", [_P, _NE * _HK * _B], F16, kind="ExternalOutput")
    ba_o = nc.dram_tensor("ba", [_P, 2 * _HK * _D], F32, kind="ExternalOutput")
    v_o = nc.dram_tensor("v", [_P, 2 * _HK], F32, kind="ExternalOutput")

    with tile.TileContext(nc) as tc:
        with (
            tc.tile_pool(name="consts", bufs=1) as cb,
            tc.tile_pool(name="one", bufs=1) as ob,
            tc.tile_pool(name="gstream", bufs=4) as gb,
            tc.tile_pool(name="sbuf", bufs=2) as sb,
            tc.tile_pool(name="esb", bufs=3) as eb,
            tc.tile_pool(name="psum", bufs=2, space="PSUM") as pp,
            tc.tile_pool(name="epsum", bufs=4, space="PSUM") as ep,
        ):
            ta = cb.tile([_P, _PK2A.total], F16)
            nc.sync.dma_start(ta[:], pk_a[:])
            tb = cb.tile([_P, _PK2B.total], F32R)
            nc.scalar.dma_start(tb[:], pk_b[:].bitcast(F32R))
            tc16 = cb.tile([_P, _PK2C.total], F16)
            nc.scalar.dma_start(tc16[:], pk_c[:])

            ident = cb.tile([_P, _P], F32)
            make_identity(nc, ident[:])

            warm_ps = ep.tile([_P, _B], F32, tag="eps")
            for _ in range(24):
                nc.tensor.matmul(
                    warm_ps[:, 0:_P], ident[:], ident[:], start=True, stop=True
                )

            fpT16 = _PK2A.view(ta, "fpT16")
            Tg = _PK2B.view(tb, "Tg")
            Ss = _PK2B.view(tb, "Ss")
            fpT = _PK2B.view(tb, "fpT")
            fhr = _PK2B.view(tb, "fhr")
            bc1 = _PK2B.view(tb, "bc1")[:, 0, :].bitcast(F32)
            bc2 = _PK2B.view(tb, "bc2")[:, 0, :].bitcast(F32)
            wc1p = _PK2C.view(tc16, "wc1p")
            wc1b = _PK2C.view(tc16, "wc1b")
            wc2 = _PK2C.view(tc16, "wc2")
            pT = _PK2C.view(tc16, "pT")
            hT = _PK2C.view(tc16, "hT")

            # ---- E row-block (7 rotated chunks), streamed; highest priority
            for j in range(_NE):
                gt = gb.tile([_P, _HK, _B], F16, tag="gchunk")
                gsrc = G_i[:, j * _HK * _B : (j + 1) * _HK * _B].rearrange(
                    "p (t n) -> p t n", t=_HK
                )
                if j == 0:
                    # halve the first chunk's completion latency: the kt 0-1
                    # matmuls can start while kt 2-3 are still in flight
                    nc.sync.dma_start(gt[:, 0:2, :], gsrc[:, 0:2, :])
                    nc.sync.dma_start(gt[:, 2:4, :], gsrc[:, 2:4, :])
                else:
                    nc.sync.dma_start(gt[:], gsrc)
                es = eb.tile([_P, _HK, _B], F16, tag="esb")
                for mt in range(_HK):
                    ps = ep.tile([_P, _B], F32, tag="eps")
                    for kt in range(_HK):
                        nc.tensor.matmul(
                            ps[:],
                            fpT16[:, kt, ts(mt, _P)],
                            gt[:, kt, :],
                            start=(kt == 0),
                            stop=(kt == _HK - 1),
                        )
                    nc.vector.tensor_copy(es[:, mt, :], ps[:])
                nc.scalar.dma_start(
                    E_o[:, j * _HK * _B : (j + 1) * _HK * _B], es[:]
                )

            ba_sb = ob.tile([_P, 2, _HK, _D], F32)

            def normalized_block(lhsT_tile, rhs_tile, slot, tag):
                """row-major block + normalize by col 300; also emit the
                feature-major f32r transpose for comp()."""
                rec = ob.tile([_P, _HK], F32, tag=f"{tag}rec")
                rm = ob.tile([_P, _HK, _DPAD], F32, tag="normrm")
                nc.vector.memset(rm[:], 0.0)
                for mt in range(_HK):
                    ps = pp.tile([_P, _DN], F32, tag="normps")
                    for kt in range(_HK):
                        nc.tensor.matmul(
                            ps[:],
                            lhsT_tile[:, kt, ts(mt, _P)],
                            rhs_tile[:, kt, :],
                            start=(kt == 0),
                            stop=(kt == _HK - 1),
                        )
                    nc.vector.reciprocal(rec[:, mt : mt + 1], ps[:, _D : _D + 1])
                    nc.vector.tensor_scalar_mul(
                        rm[:, mt, 0:_D], ps[:, 0:_D], rec[:, mt : mt + 1]
                    )
                    nc.vector.tensor_copy(ba_sb[:, slot, mt, :], rm[:, mt, 0:_D])
                tT = ob.tile([_P, _DK, _B], F16, tag=tag)
                for i in range(_HK):
                    for j in range(_DK):
                        tp = pp.tile([_P, _P], F32, tag="compps")
                        nc.tensor.transpose(tp[:], rm[:, i, ts(j, _P)], ident[:])
                        nc.vector.tensor_copy(tT[:, j, ts(i, _P)], tp[:])
                return tT

            betaT = normalized_block(fpT, Tg, 0, "betaT")
            alphaT = normalized_block(fhr, Ss, 1, "alphaT")
            nc.scalar.dma_start(ba_o[:], ba_sb[:])

            v_sb = ob.tile([_P, 2, _HK], F32)

            def comp_partial(embT, xT, slot, tag):
                z1 = ob.tile([_P, _HK, _B], F16, tag="compz1")
                for mt in range(_HK):
                    ps = pp.tile([_P, _B], F32, tag="compps")
                    for kt in range(_DK):
                        nc.tensor.matmul(
                            ps[:],
                            wc1p[:, kt, ts(mt, _P)],
                            embT[:, kt, :],
                            start=(kt == 0),
                            stop=False,
                        )
                    for kt in range(_DK):
                        nc.tensor.matmul(
                            ps[:],
                            wc1b[:, kt, ts(mt, _P)],
                            xT[:, kt, :],
                            start=False,
                            stop=(kt == _DK - 1),
                        )
                    nc.scalar.activation(
                        z1[:, mt, :],
                        ps[:],
                        mybir.ActivationFunctionType.Relu,
                        bias=bc1[:, mt : mt + 1],
                    )
                for mt in range(_HK):
                    z2 = sb.tile([_P, _B], F32, tag=f"c{tag}z2")
                    ps = pp.tile([_P, _B], F32, tag="compps")
                    for kt in range(_HK):
                        nc.tensor.matmul(
                            ps[:],
                            wc2[:, kt, ts(mt, _P)],
                            z1[:, kt, :],
                            start=(kt == 0),
                            stop=(kt == _HK - 1),
                        )
                    nc.scalar.activation(
                        z2[:],
                        ps[:],
                        mybir.ActivationFunctionType.Relu,
                        bias=bc2[:, mt : mt + 1],
                    )
                    nc.vector.reduce_sum(
                        v_sb[:, slot, mt : mt + 1], z2[:], axis=mybir.AxisListType.X
                    )

            comp_partial(pT, betaT, 0, "1")
            comp_partial(hT, alphaT, 1, "2")
            nc.scalar.dma_start(v_o[:], v_sb[:])

    nc.compile()
    return nc


def _get(name):
    if name not in _cache:
        _cache[name] = _build_l1() if name == "l1" else _build_l2()
    return _cache[name]


def kernel(
    p_idx,
    h_idx,
    emb,
    W_a1,
    b_a1,
    W_a2,
    b_a2,
    W_c1,
    b_c1,
    W_c2,
    b_c2,
    W_g1,
    b_g1,
    W_g2,
    b_g2,
    W_g3,
    b_g3,
):
    from concourse.bass_utils import run_bass_kernel_spmd

    f32 = np.float32
    emb = np.asarray(emb, f32)
    cores = list(range(_NCORES))

    # ---- shard inputs: row-lookup + slice per core ----
    p_emb = np.ascontiguousarray(emb[np.asarray(p_idx, np.int64)])  # [4096, 300]
    h_emb = np.ascontiguousarray(emb[np.asarray(h_idx, np.int64)])

    ones = np.ones((1, _B), f32)
    w1b = _pad_rows(
        np.vstack([np.asarray(W_a1, f32).T, np.asarray(b_a1, f32)[None, :]]), _DPAD
    )
    w2 = np.asarray(W_a2, f32).T
    ba2 = np.asarray(b_a2, f32).reshape(_HK, _P).T  # [128, 4]

    in_maps1 = []
    for c in range(_NCORES):
        pb = p_emb[c * _B : (c + 1) * _B]
        hs = h_emb[c::_NCORES]
        in_maps1.append(
            {
                "pk_a": _PK1A.build(
                    {
                        "w1b": w1b,
                        "ptb": _pad_rows(np.vstack([pb.T, ones]), _DPAD),
                        "pblk": pb,
                        "hblk": h_emb[c * _B : (c + 1) * _B],
                    }
                ),
                "pk_b": _PK1B.build(
                    {
                        "w2": w2,
                        "ba2": ba2,
                        "htb": _pad_rows(np.vstack([hs.T, ones]), _DPAD),
                    }
                ),
            }
        )

    res1 = run_bass_kernel_spmd(_get("l1"), in_maps1, core_ids=cores)
    LAST_RESULTS.clear()
    LAST_RESULTS.append(res1)
    r1 = res1.results

    # ---- host glue: tiny sums + assembly ----
    fpT_blocks = [_unswz(r["fpT"], _HK) for r in r1]  # [512(feat), 512(row)]
    fhT_blocks = [_unswz(r["fhT"], _HK) for r in r1]
    ST = [_unswz(r["ST"], 2 * _HK) for r in r1]  # [8*128, 300] = [S; T]
    ED = [_unswz(r["ED"], _HK) for r in r1]  # [512, 512] diag E blocks
    fh = np.empty((_L, _H), f32)
    for r in range(_NCORES):
        fh[r::_NCORES] = fhT_blocks[r].T
    G = fh.reshape(_H, _L)
    # per-chunk swizzled views of G: chunk nn -> [128, HK*B]
    G_chunks = [
        _swz(np.ascontiguousarray(G[:, nn * _B : (nn + 1) * _B]), _HK).astype(
            np.float16
        )
        for nn in range(_NCORES)
    ]
    S = np.sum([st[:_H] for st in ST], axis=0, dtype=f32)
    T = np.sum([st[_H:] for st in ST], axis=0, dtype=f32)
    sfp = np.sum([b.sum(axis=1, dtype=np.float64) for b in fpT_blocks], axis=0)
    g = G.sum(axis=1, dtype=np.float64)
    zc = np.zeros((_H, 1), f32)
    Ss = np.hstack([S, sfp[:, None].astype(f32), zc])
    Tg = np.hstack([T, g[:, None].astype(f32), zc])

    wc1p = _pad_rows(np.asarray(W_c1, f32)[:, :_D].T, _DPAD)
    wc1b = _pad_rows(np.asarray(W_c1, f32)[:, _D:].T, _DPAD)
    bc1 = np.asarray(b_c1, f32).reshape(_HK, _P).T
    wc2 = np.asarray(W_c2, f32).T
    bc2 = np.asarray(b_c2, f32).reshape(_HK, _P).T

    in_maps2 = []
    for c in range(_NCORES):
        pb = p_emb[c * _B : (c + 1) * _B]
        hb = h_emb[c * _B : (c + 1) * _B]
        perm = [(c + 1 + j) % _NCORES for j in range(_NE)]
        in_maps2.append(
            {
                "pk_a": _PK2A.build({"fpT16": fpT_blocks[c]}, np.float16),
                "pk_b": _PK2B.build(
                    {
                        "Tg": Tg,
                        "Ss": Ss,
                        "fpT": fpT_blocks[c],
                        "fhr": fhT_blocks[c].T,
                        "bc1": bc1,
                        "bc2": bc2,
                    }
                ),
                "pk_c": _PK2C.build(
                    {
                        "wc1p": wc1p,
                        "wc1b": wc1b,
                        "wc2": wc2,
                        "pT": _pad_rows(pb.T, _DPAD),
                        "hT": _pad_rows(hb.T, _DPAD),
                    },
                    np.float16,
                ),
                "G": np.ascontiguousarray(
                    np.concatenate([G_chunks[nn] for nn in perm], axis=1)
                ),
            }
        )

    res2 = run_bass_kernel_spmd(_get("l2"), in_maps2, core_ids=cores)
    LAST_RESULTS.append(res2)
    r2 = res2.results

    # ---- gather/unshard ----
    E = np.empty((_L, _L), f32)
    for c in range(_NCORES):
        rows = slice(c * _B, (c + 1) * _B)
        E[rows, c * _B : (c + 1) * _B] = ED[c]
        eflat = r2[c]["E"].astype(f32)  # [128, 7*HK*B] chunk-major
        for j in range(_NE):
            nn = (c + 1 + j) % _NCORES
            E[rows, nn * _B : (nn + 1) * _B] = _unswz(
                eflat[:, j * _HK * _B : (j + 1) * _HK * _B], _HK
            )
    ba = [_unswz(r["ba"], 2 * _HK) for r in r2]  # [8*128, 300] = [beta; alpha]
    beta = np.concatenate([b[:_H] for b in ba], axis=0)
    alpha = np.concatenate([b[_H:] for b in ba], axis=0)
    v = np.sum([r["v"] for r in r2], axis=0, dtype=f32)  # [128, 2*HK]
    v1 = v[:, :_HK].T.reshape(_H)
    v2 = v[:, _HK:].T.reshape(_H)

    # final head: [1024] -> 512 -> 512 -> 3 (tiny; host fp32)
    y = np.concatenate([v1, v2])
    y = np.maximum(y @ np.asarray(W_g1, f32).T + np.asarray(b_g1, f32), 0.0)
    y = np.maximum(y @ np.asarray(W_g2, f32).T + np.asarray(b_g2, f32), 0.0)
    y = y @ np.asarray(W_g3, f32).T + np.asarray(b_g3, f32)
    y = y - y.max()
    ey = np.exp(y)
    y = (ey / ey.sum()).astype(f32)

    return (E, beta, alpha, v1, v2, y)


# revision 26
# speedup vs baseline: 1.0391x; 1.0391x over previous
"""Trainium2 Bass kernel for nn_Discriminator (decomposable attention over
gathered embeddings).

Math (reference):
    p_emb = emb[p_idx]; h_emb = emb[h_idx]                # [4096, 300]
    fp = attend(p_emb); fh = attend(h_emb)                # [4096, 512]
    G  = fh.reshape(512, 4096)      (row-major reshape)
    E  = fp @ G                                           # [4096, 4096]
    eik = E.sum(1); ekj = E.sum(0)
    beta  = (E/eik) @ h_emb;  alpha = (E/ekj).T @ p_emb   # [4096, 300]
    v1 = comp([p_emb|beta]).sum(0); v2 = comp([h_emb|alpha]).sum(0)
    y  = softmax(mlp([v1|v2]))                            # [3]

Key identities used to shard across 8 cores without collectives
(G[k, r*512+c] == fh[8k+r, c], so G's column block r is fh[r::8]):
    eik = fp @ g,          g = G.sum(1)
    E @ h_emb = fp @ T,    T = sum_r fh[r::8] @ h_emb[r*512:(r+1)*512]
    ekj[r*512+c] = (fh[r::8].T @ sfp)[c],   sfp = fp.sum(0)
    (E.T @ p_emb)[r*512:(r+1)*512] = fh[r::8].T @ S,   S = fp.T @ p_emb

Two SPMD launches on cores 0-7:
    L1: per-core attend on its p-block (rows c*512:(c+1)*512) and its strided
        h-slice (rows r::8); partial S_c, T_r; the diagonal E block
        fp_c @ fh_r (core c owns both operands).  Host sums S/T (tiny) and
        assembles G.
    L2: per-core remaining 7 E column chunks of its row block (G pack is
        rotated per core so the single NEFF stays SPMD), beta/alpha blocks,
        v1/v2 partials.
Host does only O(KB) glue plus the final 3-way MLP head on [v1|v2].

All device inputs/outputs are pre-swizzled on the host into [128, N]
partition-major flats so every DMA is one contiguous line per partition
(sequencer descriptor-generation cost was a profiled bottleneck), and input
packs are ordered so first-needed operands complete first (DMA bandwidth is
the other profiled bottleneck).
"""

import numpy as np

_P = 128
_D = 300
_H = 512
_L = 4096
_B = 512  # rows per core
_NCORES = 8
_DPAD = 384  # 300 padded up to 3*128 (row 300 carries the ones/bias trick)
_DN = _D + 2  # fp32r needs an even moving dim; col 300 = normalizer, 301 pad

_HK = _H // _P  # 4
_DK = _DPAD // _P  # 3
_NE = _NCORES - 1  # 7 off-diagonal E column chunks in L2

_cache = {}
LAST_RESULTS = []  # BassKernelResults of the most recent kernel() launches


def _swz(a, t):
    """[t*128, n] row-major -> [128, t*n] partition-major flat."""
    n = a.shape[1]
    return a.reshape(t, _P, n).transpose(1, 0, 2).reshape(_P, t * n)


def _unswz(a, t):
    """[128, t*n] partition-major flat -> [t*128, n] row-major."""
    n = a.shape[1] // t
    return a.reshape(_P, t, n).transpose(1, 0, 2).reshape(t * _P, n)


def _pad_rows(a, rows):
    out = np.zeros((rows, a.shape[1]), np.float32)
    out[: a.shape[0]] = a
    return out


class _Pack:
    """Host-side [128, N] pack builder + device-side view registry."""

    def __init__(self):
        self.specs = []  # (name, off, t, n)
        self.total = 0

    def add(self, name, t, n):
        self.specs.append((name, self.total, t, n))
        self.total += t * n

    def off(self, name):
        for nm, off, t, n in self.specs:
            if nm == name:
                return off, t, n
        raise KeyError(name)

    def view(self, tile, name):
        off, t, n = self.off(name)
        return tile[:, off : off + t * n].rearrange("p (t n) -> p t n", t=t)

    def range_of(self, names):
        offs = []
        for nm in names:
            off, t, n = self.off(nm)
            offs.append((off, off + t * n))
        lo = min(o for o, _ in offs)
        hi = max(e for _, e in offs)
        assert hi - lo == sum(e - o for o, e in offs), "group must be contiguous"
        return lo, hi

    def build(self, arrays, dtype=np.float32):
        """arrays: {name: [t*128, n] array}; returns [128, total] in dtype."""
        out = np.empty((_P, self.total), dtype)
        for nm, off, t, n in self.specs:
            a = arrays[nm]
            assert a.shape == (t * _P, n), (nm, a.shape, (t * _P, n))
            out[:, off : off + t * n] = _swz(
                np.ascontiguousarray(a).astype(dtype), t
            )
        return out


# ---- pack layouts (module-level so host and builder agree) ----
_PK1A = _Pack()  # L1 sync: attend-p operands, then S/T rhs
_PK1A.add("w1b", _DK, _H)
_PK1A.add("ptb", _DK, _B)
_PK1A.add("pblk", _HK, _D)
_PK1A.add("hblk", _HK, _D)
_PK1B = _Pack()  # L1 scalar: attend layer-2 weights + attend-h input
_PK1B.add("w2", _HK, _H)
_PK1B.add("ba2", 1, _HK)
_PK1B.add("htb", _DK, _B)

_PK2A = _Pack()  # L2 sync fp16, ahead of the G chunks: E lhsT
_PK2A.add("fpT16", _HK, _B)
_PK2B = _Pack()  # L2 scalar f32: beta/alpha operands (f32r quality path)
_PK2B.add("Tg", _HK, _DN)
_PK2B.add("Ss", _HK, _DN)
_PK2B.add("fpT", _HK, _B)
_PK2B.add("fhr", _HK, _H)
_PK2B.add("bc1", 1, _HK)
_PK2B.add("bc2", 1, _HK)
_PK2C = _Pack()  # L2 scalar fp16: comp operands
_PK2C.add("wc1p", _DK, _H)
_PK2C.add("wc1b", _DK, _H)
_PK2C.add("wc2", _HK, _H)
_PK2C.add("pT", _DK, _B)
_PK2C.add("hT", _DK, _B)


def _build_l1():
    import concourse.bacc as bacc
    import concourse.bass as bass
    import concourse.mybir as mybir
    import concourse.tile as tile
    from concourse.masks import make_identity

    F32 = mybir.dt.float32
    F32R = mybir.dt.float32r
    ts = bass.ts

    nc = bacc.Bacc("TRN2", target_bir_lowering=False, debug=False, num_devices=_NCORES)

    pk_a = nc.dram_tensor("pk_a", [_P, _PK1A.total], F32, kind="ExternalInput")
    pk_b = nc.dram_tensor("pk_b", [_P, _PK1B.total], F32, kind="ExternalInput")

    fpT_o = nc.dram_tensor("fpT", [_P, _HK * _B], F32, kind="ExternalOutput")
    fhT_o = nc.dram_tensor("fhT", [_P, _HK * _B], F32, kind="ExternalOutput")
    ST_o = nc.dram_tensor("ST", [_P, 2 * _HK * _D], F32, kind="ExternalOutput")
    ED_o = nc.dram_tensor("ED", [_P, _HK * _B], F32, kind="ExternalOutput")

    with tile.TileContext(nc) as tc:
        with (
            tc.tile_pool(name="consts", bufs=1) as cb,
            tc.tile_pool(name="one", bufs=1) as ob,
            tc.tile_pool(name="sbuf", bufs=2) as sb,
            tc.tile_pool(name="psum", bufs=2, space="PSUM") as pp,
            tc.tile_pool(name="edpsum", bufs=2, space="PSUM") as ep,
        ):
            ta = cb.tile([_P, _PK1A.total], F32R)
            lo, hi = _PK1A.range_of(["w1b", "ptb"])
            nc.sync.dma_start(ta[:, lo:hi], pk_a[:, lo:hi].bitcast(F32R))
            lo2, hi2 = _PK1A.range_of(["pblk", "hblk"])
            nc.sync.dma_start(ta[:, lo2:hi2], pk_a[:, lo2:hi2].bitcast(F32R))
            tb = cb.tile([_P, _PK1B.total], F32R)
            nc.scalar.dma_start(tb[:], pk_b[:].bitcast(F32R))

            ident = cb.tile([_P, _P], F32)
            make_identity(nc, ident[:])

            # PE pre-warm: ~4us of junk matmuls during the input-DMA wait so
            # the HAM clock gate opens (1.2 -> 2.4 GHz) before real work.
            warm_ps = pp.tile([_P, _B], F32, tag="attps")
            for _ in range(24):
                nc.tensor.matmul(
                    warm_ps[:, 0:_P], ident[:], ident[:], start=True, stop=True
                )

            w1b_t = _PK1A.view(ta, "w1b")
            ptb_t = _PK1A.view(ta, "ptb")
            pblk_t = _PK1A.view(ta, "pblk")
            hblk_t = _PK1A.view(ta, "hblk")
            w2_t = _PK1B.view(tb, "w2")
            ba2_t = _PK1B.view(tb, "ba2")[:, 0, :].bitcast(F32)
            htb_t = _PK1B.view(tb, "htb")

            def attend_T(xt):
                z1 = sb.tile([_P, _HK, _B], F32R, tag="attz1")
                for mt in range(_HK):
                    ps = pp.tile([_P, _B], F32, tag="attps")
                    for kt in range(_DK):
                        nc.tensor.matmul(
                            ps[:],
                            w1b_t[:, kt, ts(mt, _P)],
                            xt[:, kt, :],
                            start=(kt == 0),
                            stop=(kt == _DK - 1),
                        )
                    nc.scalar.activation(
                        z1[:, mt, :], ps[:], mybir.ActivationFunctionType.Relu
                    )
                fT = sb.tile([_P, _HK, _B], F32R, tag="attout")
                for mt in range(_HK):
                    ps = pp.tile([_P, _B], F32, tag="attps")
                    for kt in range(_HK):
                        nc.tensor.matmul(
                            ps[:],
                            w2_t[:, kt, ts(mt, _P)],
                            z1[:, kt, :],
                            start=(kt == 0),
                            stop=(kt == _HK - 1),
                        )
                    nc.scalar.activation(
                        fT[:, mt, :],
                        ps[:],
                        mybir.ActivationFunctionType.Relu,
                        bias=ba2_t[:, mt : mt + 1],
                    )
                return fT

            fpT = attend_T(ptb_t)
            nc.sync.dma_start(fpT_o[:].bitcast(F32R), fpT[:])
            fhT = attend_T(htb_t)
            nc.scalar.dma_start(fhT_o[:].bitcast(F32R), fhT[:])

            def transpose_16(src):
                rm = ob.tile([_P, _HK, _H], F32R, tag=f"rm{src is fhT}")
                for i in range(_HK):
                    for j in range(_HK):
                        tp = pp.tile([_P, _P], F32, tag="tps")
                        nc.tensor.transpose(
                            tp[:], src[:, i, ts(j, _P)].bitcast(F32), ident[:]
                        )
                        nc.vector.tensor_copy(rm[:, j, ts(i, _P)], tp[:].bitcast(F32R))
                return rm

            st_sb = ob.tile([_P, 2, _HK, _D], F32)
            # T_r[k, d] = sum_c fh_r[k, c] * h_blk[c, d]  (lhsT = fhT directly)
            for mt in range(_HK):
                ps = pp.tile([_P, _D], F32, tag="stps")
                for kt in range(_HK):
                    nc.tensor.matmul(
                        ps[:],
                        fhT[:, kt, ts(mt, _P)],
                        hblk_t[:, kt, :],
                        start=(kt == 0),
                        stop=(kt == _HK - 1),
                    )
                nc.vector.tensor_copy(st_sb[:, 1, mt, :], ps[:])
            nc.scalar.dma_start(ST_o[:, _HK * _D :], st_sb[:, 1, :, :])

            fh_rm = transpose_16(fhT)  # fh_r row-major: E diag chunk's rhs

            # E diagonal block: fp_c @ G[:, r*512:(r+1)*512] = fp_c @ fh_r
            ed = ob.tile([_P, _HK, _B], F32)
            for mt in range(_HK):
                ps = ep.tile([_P, _B], F32, tag="edps")
                for kt in range(_HK):
                    nc.tensor.matmul(
                        ps[:],
                        fpT[:, kt, ts(mt, _P)],
                        fh_rm[:, kt, :],
                        start=(kt == 0),
                        stop=(kt == _HK - 1),
                    )
                nc.vector.tensor_copy(ed[:, mt, :], ps[:])
            nc.scalar.dma_start(ED_o[:], ed[:])

            fp_rm = transpose_16(fpT)  # fp row-major: S's lhsT

            # S_c[k, d] = sum_i fp[i, k] * p_emb[i, d]
            for mt in range(_HK):
                ps = pp.tile([_P, _D], F32, tag="stps")
                for kt in range(_HK):
                    nc.tensor.matmul(
                        ps[:],
                        fp_rm[:, kt, ts(mt, _P)],
                        pblk_t[:, kt, :],
                        start=(kt == 0),
                        stop=(kt == _HK - 1),
                    )
                nc.vector.tensor_copy(st_sb[:, 0, mt, :], ps[:])
            nc.sync.dma_start(ST_o[:, : _HK * _D], st_sb[:, 0, :, :])

    nc.compile()
    return nc


def _build_l2():
    import concourse.bacc as bacc
    import concourse.bass as bass
    import concourse.mybir as mybir
    import concourse.tile as tile
    from concourse.masks import make_identity

    F32 = mybir.dt.float32
    F32R = mybir.dt.float32r
    F16 = mybir.dt.float16
    ts = bass.ts

    nc = bacc.Bacc("TRN2", target_bir_lowering=False, debug=False, num_devices=_NCORES)

    pk_a = nc.dram_tensor("pk_a", [_P, _PK2A.total], F16, kind="ExternalInput")
    pk_b = nc.dram_tensor("pk_b", [_P, _PK2B.total], F32, kind="ExternalInput")
    pk_c = nc.dram_tensor("pk_c", [_P, _PK2C.total], F16, kind="ExternalInput")
    # G packed chunk-major, 7 per-core-rotated chunks: [p][j][kt][512]
    G_i = nc.dram_tensor("G", [_P, _NE * _HK * _B], F16, kind="ExternalInput")

    # E packed chunk-major [p][j][mt][512]; host unswizzles + unrotates
    E_o = nc.dram_tensor("E", [_P, _NE * _HK * _B], F16, kind="ExternalOutput")
    ba_o = nc.dram_tensor("ba", [_P, 2 * _HK * _D], F32, kind="ExternalOutput")
    v_o = nc.dram_tensor("v", [_P, 2 * _HK], F32, kind="ExternalOutput")

    with tile.TileContext(nc) as tc:
        with (
            tc.tile_pool(name="consts", bufs=1) as cb,
            tc.tile_pool(name="one", bufs=1) as ob,
            tc.tile_pool(name="gstream", bufs=4) as gb,
            tc.tile_pool(name="sbuf", bufs=2) as sb,
            tc.tile_pool(name="esb", bufs=3) as eb,
            tc.tile_pool(name="psum", bufs=2, space="PSUM") as pp,
            tc.tile_pool(name="epsum", bufs=4, space="PSUM") as ep,
        ):
            ta = cb.tile([_P, _PK2A.total], F16)
            nc.sync.dma_start(ta[:], pk_a[:])
            tb = cb.tile([_P, _PK2B.total], F32R)
            nc.scalar.dma_start(tb[:], pk_b[:].bitcast(F32R))
            tc16 = cb.tile([_P, _PK2C.total], F16)
            nc.scalar.dma_start(tc16[:], pk_c[:])

            ident = cb.tile([_P, _P], F32)
            make_identity(nc, ident[:])

            warm_ps = pp.tile([_P, _B], F32, tag="compps")
            for _ in range(24):
                nc.tensor.matmul(
                    warm_ps[:, 0:_P], ident[:], ident[:], start=True, stop=True
                )

            fpT16 = _PK2A.view(ta, "fpT16")
            Tg = _PK2B.view(tb, "Tg")
            Ss = _PK2B.view(tb, "Ss")
            fpT = _PK2B.view(tb, "fpT")
            fhr = _PK2B.view(tb, "fhr")
            bc1 = _PK2B.view(tb, "bc1")[:, 0, :].bitcast(F32)
            bc2 = _PK2B.view(tb, "bc2")[:, 0, :].bitcast(F32)
            wc1p = _PK2C.view(tc16, "wc1p")
            wc1b = _PK2C.view(tc16, "wc1b")
            wc2 = _PK2C.view(tc16, "wc2")
            pT = _PK2C.view(tc16, "pT")
            hT = _PK2C.view(tc16, "hT")

            # ---- E row-block (7 rotated chunks), streamed; highest priority
            for j in range(_NE):
                gt = gb.tile([_P, _HK, _B], F16, tag="gchunk")
                nc.sync.dma_start(
                    gt[:],
                    G_i[:, j * _HK * _B : (j + 1) * _HK * _B].rearrange(
                        "p (t n) -> p t n", t=_HK
                    ),
                )
                es = eb.tile([_P, _HK, _B], F16, tag="esb")
                for mt in range(_HK):
                    ps = ep.tile([_P, _B], F32, tag="eps")
                    for kt in range(_HK):
                        nc.tensor.matmul(
                            ps[:],
                            fpT16[:, kt, ts(mt, _P)],
                            gt[:, kt, :],
                            start=(kt == 0),
                            stop=(kt == _HK - 1),
                        )
                    nc.vector.tensor_copy(es[:, mt, :], ps[:])
                nc.scalar.dma_start(
                    E_o[:, j * _HK * _B : (j + 1) * _HK * _B], es[:]
                )

            ba_sb = ob.tile([_P, 2, _HK, _D], F32)

            def normalized_block(lhsT_tile, rhs_tile, slot, tag):
                """row-major block + normalize by col 300; also emit the
                feature-major f32r transpose for comp()."""
                rec = ob.tile([_P, _HK], F32, tag=f"{tag}rec")
                rm = ob.tile([_P, _HK, _DPAD], F32, tag="normrm")
                nc.vector.memset(rm[:], 0.0)
                for mt in range(_HK):
                    ps = pp.tile([_P, _DN], F32, tag="normps")
                    for kt in range(_HK):
                        nc.tensor.matmul(
                            ps[:],
                            lhsT_tile[:, kt, ts(mt, _P)],
                            rhs_tile[:, kt, :],
                            start=(kt == 0),
                            stop=(kt == _HK - 1),
                        )
                    nc.vector.reciprocal(rec[:, mt : mt + 1], ps[:, _D : _D + 1])
                    nc.vector.tensor_scalar_mul(
                        rm[:, mt, 0:_D], ps[:, 0:_D], rec[:, mt : mt + 1]
                    )
                    nc.vector.tensor_copy(ba_sb[:, slot, mt, :], rm[:, mt, 0:_D])
                tT = ob.tile([_P, _DK, _B], F16, tag=tag)
                for i in range(_HK):
                    for j in range(_DK):
                        tp = pp.tile([_P, _P], F32, tag="compps")
                        nc.tensor.transpose(tp[:], rm[:, i, ts(j, _P)], ident[:])
                        nc.vector.tensor_copy(tT[:, j, ts(i, _P)], tp[:])
                return tT

            betaT = normalized_block(fpT, Tg, 0, "betaT")
            alphaT = normalized_block(fhr, Ss, 1, "alphaT")
            nc.scalar.dma_start(ba_o[:], ba_sb[:])

            v_sb = ob.tile([_P, 2, _HK], F32)

            def comp_partial(embT, xT, slot, tag):
                z1 = ob.tile([_P, _HK, _B], F16, tag="compz1")
                for mt in range(_HK):
                    ps = pp.tile([_P, _B], F32, tag="compps")
                    for kt in range(_DK):
                        nc.tensor.matmul(
                            ps[:],
                            wc1p[:, kt, ts(mt, _P)],
                            embT[:, kt, :],
                            start=(kt == 0),
                            stop=False,
                        )
                    for kt in range(_DK):
                        nc.tensor.matmul(
                            ps[:],
                            wc1b[:, kt, ts(mt, _P)],
                            xT[:, kt, :],
                            start=False,
                            stop=(kt == _DK - 1),
                        )
                    nc.scalar.activation(
                        z1[:, mt, :],
                        ps[:],
                        mybir.ActivationFunctionType.Relu,
                        bias=bc1[:, mt : mt + 1],
                    )
                for mt in range(_HK):
                    z2 = sb.tile([_P, _B], F32, tag=f"c{tag}z2")
                    ps = pp.tile([_P, _B], F32, tag="compps")
                    for kt in range(_HK):
                        nc.tensor.matmul(
                            ps[:],
                            wc2[:, kt, ts(mt, _P)],
                            z1[:, kt, :],
                            start=(kt == 0),
                            stop=(kt == _HK - 1),
                        )
                    nc.scalar.activation(
                        z2[:],
                        ps[:],
                        mybir.ActivationFunctionType.Relu,
                        bias=bc2[:, mt : mt + 1],
                    )
                    nc.vector.reduce_sum(
                        v_sb[:, slot, mt : mt + 1], z2[:], axis=mybir.AxisListType.X
                    )

            comp_partial(pT, betaT, 0, "1")
            comp_partial(hT, alphaT, 1, "2")
            nc.scalar.dma_start(v_o[:], v_sb[:])

    nc.compile()
    return nc


def _get(name):
    if name not in _cache:
        _cache[name] = _build_l1() if name == "l1" else _build_l2()
    return _cache[name]


def kernel(
    p_idx,
    h_idx,
    emb,
    W_a1,
    b_a1,
    W_a2,
    b_a2,
    W_c1,
    b_c1,
    W_c2,
    b_c2,
    W_g1,
    b_g1,
    W_g2,
    b_g2,
    W_g3,
    b_g3,
):
    from concourse.bass_utils import run_bass_kernel_spmd

    f32 = np.float32
    emb = np.asarray(emb, f32)
    cores = list(range(_NCORES))

    # ---- shard inputs: row-lookup + slice per core ----
    p_emb = np.ascontiguousarray(emb[np.asarray(p_idx, np.int64)])  # [4096, 300]
    h_emb = np.ascontiguousarray(emb[np.asarray(h_idx, np.int64)])

    ones = np.ones((1, _B), f32)
    w1b = _pad_rows(
        np.vstack([np.asarray(W_a1, f32).T, np.asarray(b_a1, f32)[None, :]]), _DPAD
    )
    w2 = np.asarray(W_a2, f32).T
    ba2 = np.asarray(b_a2, f32).reshape(_HK, _P).T  # [128, 4]

    in_maps1 = []
    for c in range(_NCORES):
        pb = p_emb[c * _B : (c + 1) * _B]
        hs = h_emb[c::_NCORES]
        in_maps1.append(
            {
                "pk_a": _PK1A.build(
                    {
                        "w1b": w1b,
                        "ptb": _pad_rows(np.vstack([pb.T, ones]), _DPAD),
                        "pblk": pb,
                        "hblk": h_emb[c * _B : (c + 1) * _B],
                    }
                ),
                "pk_b": _PK1B.build(
                    {
                        "w2": w2,
                        "ba2": ba2,
                        "htb": _pad_rows(np.vstack([hs.T, ones]), _DPAD),
                    }
                ),
            }
        )

    res1 = run_bass_kernel_spmd(_get("l1"), in_maps1, core_ids=cores)
    LAST_RESULTS.clear()
    LAST_RESULTS.append(res1)
    r1 = res1.results

    # ---- host glue: tiny sums + assembly ----
    fpT_blocks = [_unswz(r["fpT"], _HK) for r in r1]  # [512(feat), 512(row)]
    fhT_blocks = [_unswz(r["fhT"], _HK) for r in r1]
    ST = [_unswz(r["ST"], 2 * _HK) for r in r1]  # [8*128, 300] = [S; T]
    ED = [_unswz(r["ED"], _HK) for r in r1]  # [512, 512] diag E blocks
    fh = np.empty((_L, _H), f32)
    for r in range(_NCORES):
        fh[r::_NCORES] = fhT_blocks[r].T
    G = fh.reshape(_H, _L)
    # per-chunk swizzled views of G: chunk nn -> [128, HK*B]
    G_chunks = [
        _swz(np.ascontiguousarray(G[:, nn * _B : (nn + 1) * _B]), _HK).astype(
            np.float16
        )
        for nn in range(_NCORES)
    ]
    S = np.sum([st[:_H] for st in ST], axis=0, dtype=f32)
    T = np.sum([st[_H:] for st in ST], axis=0, dtype=f32)
    sfp = np.sum([b.sum(axis=1, dtype=np.float64) for b in fpT_blocks], axis=0)
    g = G.sum(axis=1, dtype=np.float64)
    zc = np.zeros((_H, 1), f32)
    Ss = np.hstack([S, sfp[:, None].astype(f32), zc])
    Tg = np.hstack([T, g[:, None].astype(f32), zc])

    wc1p = _pad_rows(np.asarray(W_c1, f32)[:, :_D].T, _DPAD)
    wc1b = _pad_rows(np.asarray(W_c1, f32)[:, _D:].T, _DPAD)
    bc1 = np.asarray(b_c1, f32).reshape(_HK, _P).T
    wc2 = np.asarray(W_c2, f32).T
    bc2 = np.asarray(b_c2, f32).reshape(_HK, _P).T

    in_maps2 = []
    for c in range(_NCORES):
        pb = p_emb[c * _B : (c + 1) * _B]
        hb = h_emb[c * _B : (c + 1) * _B]
        perm = [(c + 1 + j) % _NCORES for j in range(_NE)]
        in_maps2.append(
            {
                "pk_a": _PK2A.build({"fpT16": fpT_blocks[c]}, np.float16),
                "pk_b": _PK2B.build(
                    {
                        "Tg": Tg,
                        "Ss": Ss,
                        "fpT": fpT_blocks[c],
                        "fhr": fhT_blocks[c].T,
                        "bc1": bc1,
                        "bc2": bc2,
                    }
                ),
                "pk_c": _PK2C.build(
                    {
                        "wc1p": wc1p,
                        "wc1b": wc1b,
                        "wc2": wc2,
                        "pT": _pad_rows(pb.T, _DPAD),
                        "hT": _pad_rows(hb.T, _DPAD),
                    },
                    np.float16,
                ),
                "G": np.ascontiguousarray(
                    np.concatenate([G_chunks[nn] for nn in perm], axis=1)
                ),
            }
        )

    res2 = run_bass_kernel_spmd(_get("l2"), in_maps2, core_ids=cores)
    LAST_RESULTS.append(res2)
    r2 = res2.results

    # ---- gather/unshard ----
    E = np.empty((_L, _L), f32)
    for c in range(_NCORES):
        rows = slice(c * _B, (c + 1) * _B)
        E[rows, c * _B : (c + 1) * _B] = ED[c]
        eflat = r2[c]["E"].astype(f32)  # [128, 7*HK*B] chunk-major
        for j in range(_NE):
            nn = (c + 1 + j) % _NCORES
            E[rows, nn * _B : (nn + 1) * _B] = _unswz(
                eflat[:, j * _HK * _B : (j + 1) * _HK * _B], _HK
            )
    ba = [_unswz(r["ba"], 2 * _HK) for r in r2]  # [8*128, 300] = [beta; alpha]
    beta = np.concatenate([b[:_H] for b in ba], axis=0)
    alpha = np.concatenate([b[_H:] for b in ba], axis=0)
    v = np.sum([r["v"] for r in r2], axis=0, dtype=f32)  # [128, 2*HK]
    v1 = v[:, :_HK].T.reshape(_H)
    v2 = v[:, _HK:].T.reshape(_H)

    # final head: [1024] -> 512 -> 512 -> 3 (tiny; host fp32)
    y = np.concatenate([v1, v2])
    y = np.maximum(y @ np.asarray(W_g1, f32).T + np.asarray(b_g1, f32), 0.0)
    y = np.maximum(y @ np.asarray(W_g2, f32).T + np.asarray(b_g2, f32), 0.0)
    y = y @ np.asarray(W_g3, f32).T + np.asarray(b_g3, f32)
    y = y - y.max()
    ey = np.exp(y)
    y = (ey / ey.sum()).astype(f32)

    return (E, beta, alpha, v1, v2, y)


# revision 27
# speedup vs baseline: 1.0405x; 1.0013x over previous
"""Trainium2 Bass kernel for nn_Discriminator (decomposable attention over
gathered embeddings).

Math (reference):
    p_emb = emb[p_idx]; h_emb = emb[h_idx]                # [4096, 300]
    fp = attend(p_emb); fh = attend(h_emb)                # [4096, 512]
    G  = fh.reshape(512, 4096)      (row-major reshape)
    E  = fp @ G                                           # [4096, 4096]
    eik = E.sum(1); ekj = E.sum(0)
    beta  = (E/eik) @ h_emb;  alpha = (E/ekj).T @ p_emb   # [4096, 300]
    v1 = comp([p_emb|beta]).sum(0); v2 = comp([h_emb|alpha]).sum(0)
    y  = softmax(mlp([v1|v2]))                            # [3]

Key identities used to shard across 8 cores without collectives
(G[k, r*512+c] == fh[8k+r, c], so G's column block r is fh[r::8]):
    eik = fp @ g,          g = G.sum(1)
    E @ h_emb = fp @ T,    T = sum_r fh[r::8] @ h_emb[r*512:(r+1)*512]
    ekj[r*512+c] = (fh[r::8].T @ sfp)[c],   sfp = fp.sum(0)
    (E.T @ p_emb)[r*512:(r+1)*512] = fh[r::8].T @ S,   S = fp.T @ p_emb

Two SPMD launches on cores 0-7:
    L1: per-core attend on its p-block (rows c*512:(c+1)*512) and its strided
        h-slice (rows r::8); partial S_c, T_r; the diagonal E block
        fp_c @ fh_r (core c owns both operands).  Host sums S/T (tiny) and
        assembles G.
    L2: per-core remaining 7 E column chunks of its row block (G pack is
        rotated per core so the single NEFF stays SPMD), beta/alpha blocks,
        v1/v2 partials.
Host does only O(KB) glue plus the final 3-way MLP head on [v1|v2].

All device inputs/outputs are pre-swizzled on the host into [128, N]
partition-major flats so every DMA is one contiguous line per partition
(sequencer descriptor-generation cost was a profiled bottleneck), and input
packs are ordered so first-needed operands complete first (DMA bandwidth is
the other profiled bottleneck).
"""

import numpy as np

_P = 128
_D = 300
_H = 512
_L = 4096
_B = 512  # rows per core
_NCORES = 8
_DPAD = 384  # 300 padded up to 3*128 (row 300 carries the ones/bias trick)
_DN = _D + 2  # fp32r needs an even moving dim; col 300 = normalizer, 301 pad

_HK = _H // _P  # 4
_DK = _DPAD // _P  # 3
_NE = _NCORES - 1  # 7 off-diagonal E column chunks in L2

_cache = {}
LAST_RESULTS = []  # BassKernelResults of the most recent kernel() launches


def _swz(a, t):
    """[t*128, n] row-major -> [128, t*n] partition-major flat."""
    n = a.shape[1]
    return a.reshape(t, _P, n).transpose(1, 0, 2).reshape(_P, t * n)


def _unswz(a, t):
    """[128, t*n] partition-major flat -> [t*128, n] row-major."""
    n = a.shape[1] // t
    return a.reshape(_P, t, n).transpose(1, 0, 2).reshape(t * _P, n)


def _pad_rows(a, rows):
    out = np.zeros((rows, a.shape[1]), np.float32)
    out[: a.shape[0]] = a
    return out


class _Pack:
    """Host-side [128, N] pack builder + device-side view registry."""

    def __init__(self):
        self.specs = []  # (name, off, t, n)
        self.total = 0

    def add(self, name, t, n):
        self.specs.append((name, self.total, t, n))
        self.total += t * n

    def off(self, name):
        for nm, off, t, n in self.specs:
            if nm == name:
                return off, t, n
        raise KeyError(name)

    def view(self, tile, name):
        off, t, n = self.off(name)
        return tile[:, off : off + t * n].rearrange("p (t n) -> p t n", t=t)

    def range_of(self, names):
        offs = []
        for nm in names:
            off, t, n = self.off(nm)
            offs.append((off, off + t * n))
        lo = min(o for o, _ in offs)
        hi = max(e for _, e in offs)
        assert hi - lo == sum(e - o for o, e in offs), "group must be contiguous"
        return lo, hi

    def build(self, arrays, dtype=np.float32):
        """arrays: {name: [t*128, n] array}; returns [128, total] in dtype."""
        out = np.empty((_P, self.total), dtype)
        for nm, off, t, n in self.specs:
            a = arrays[nm]
            assert a.shape == (t * _P, n), (nm, a.shape, (t * _P, n))
            out[:, off : off + t * n] = _swz(
                np.ascontiguousarray(a).astype(dtype), t
            )
        return out


# ---- pack layouts (module-level so host and builder agree) ----
_PK1A = _Pack()  # L1 sync: attend-p operands, then S/T rhs
_PK1A.add("w1b", _DK, _H)
_PK1A.add("ptb", _DK, _B)
_PK1A.add("pblk", _HK, _D)
_PK1A.add("hblk", _HK, _D)
_PK1B = _Pack()  # L1 scalar: attend layer-2 weights + attend-h input
_PK1B.add("w2", _HK, _H)
_PK1B.add("ba2", 1, _HK)
_PK1B.add("htb", _DK, _B)

_PK2A = _Pack()  # L2 sync fp16, ahead of the G chunks: E lhsT
_PK2A.add("fpT16", _HK, _B)
_PK2B = _Pack()  # L2 scalar f32: beta/alpha operands (f32r quality path)
_PK2B.add("Tg", _HK, _DN)
_PK2B.add("Ss", _HK, _DN)
_PK2B.add("fpT", _HK, _B)
_PK2B.add("fhr", _HK, _H)
_PK2B.add("bc1", 1, _HK)
_PK2B.add("bc2", 1, _HK)
_PK2C = _Pack()  # L2 scalar fp16: comp operands
_PK2C.add("wc1p", _DK, _H)
_PK2C.add("wc1b", _DK, _H)
_PK2C.add("wc2", _HK, _H)
_PK2C.add("pT", _DK, _B)
_PK2C.add("hT", _DK, _B)


def _build_l1():
    import concourse.bacc as bacc
    import concourse.bass as bass
    import concourse.mybir as mybir
    import concourse.tile as tile
    from concourse.masks import make_identity

    F32 = mybir.dt.float32
    F32R = mybir.dt.float32r
    ts = bass.ts

    nc = bacc.Bacc("TRN2", target_bir_lowering=False, debug=False, num_devices=_NCORES)

    pk_a = nc.dram_tensor("pk_a", [_P, _PK1A.total], F32, kind="ExternalInput")
    pk_b = nc.dram_tensor("pk_b", [_P, _PK1B.total], F32, kind="ExternalInput")

    fpT_o = nc.dram_tensor("fpT", [_P, _HK * _B], F32, kind="ExternalOutput")
    fhT_o = nc.dram_tensor("fhT", [_P, _HK * _B], F32, kind="ExternalOutput")
    ST_o = nc.dram_tensor("ST", [_P, 2 * _HK * _D], F32, kind="ExternalOutput")
    ED_o = nc.dram_tensor("ED", [_P, _HK * _B], F32, kind="ExternalOutput")

    with tile.TileContext(nc) as tc:
        with (
            tc.tile_pool(name="consts", bufs=1) as cb,
            tc.tile_pool(name="one", bufs=1) as ob,
            tc.tile_pool(name="sbuf", bufs=2) as sb,
            tc.tile_pool(name="psum", bufs=2, space="PSUM") as pp,
            tc.tile_pool(name="edpsum", bufs=2, space="PSUM") as ep,
        ):
            ta = cb.tile([_P, _PK1A.total], F32R)
            lo, hi = _PK1A.range_of(["w1b", "ptb"])
            nc.sync.dma_start(ta[:, lo:hi], pk_a[:, lo:hi].bitcast(F32R))
            lo2, hi2 = _PK1A.range_of(["pblk", "hblk"])
            nc.sync.dma_start(ta[:, lo2:hi2], pk_a[:, lo2:hi2].bitcast(F32R))
            tb = cb.tile([_P, _PK1B.total], F32R)
            nc.scalar.dma_start(tb[:], pk_b[:].bitcast(F32R))

            ident = cb.tile([_P, _P], F32)
            make_identity(nc, ident[:])

            # PE pre-warm: ~4us of junk matmuls during the input-DMA wait so
            # the HAM clock gate opens (1.2 -> 2.4 GHz) before real work.
            warm_ps = pp.tile([_P, _B], F32, tag="attps")
            for _ in range(24):
                nc.tensor.matmul(
                    warm_ps[:, 0:_P], ident[:], ident[:], start=True, stop=True
                )

            w1b_t = _PK1A.view(ta, "w1b")
            ptb_t = _PK1A.view(ta, "ptb")
            pblk_t = _PK1A.view(ta, "pblk")
            hblk_t = _PK1A.view(ta, "hblk")
            w2_t = _PK1B.view(tb, "w2")
            ba2_t = _PK1B.view(tb, "ba2")[:, 0, :].bitcast(F32)
            htb_t = _PK1B.view(tb, "htb")

            def attend_T(xt):
                z1 = sb.tile([_P, _HK, _B], F32R, tag="attz1")
                for mt in range(_HK):
                    ps = pp.tile([_P, _B], F32, tag="attps")
                    for kt in range(_DK):
                        nc.tensor.matmul(
                            ps[:],
                            w1b_t[:, kt, ts(mt, _P)],
                            xt[:, kt, :],
                            start=(kt == 0),
                            stop=(kt == _DK - 1),
                        )
                    nc.scalar.activation(
                        z1[:, mt, :], ps[:], mybir.ActivationFunctionType.Relu
                    )
                fT = sb.tile([_P, _HK, _B], F32R, tag="attout")
                for mt in range(_HK):
                    ps = pp.tile([_P, _B], F32, tag="attps")
                    for kt in range(_HK):
                        nc.tensor.matmul(
                            ps[:],
                            w2_t[:, kt, ts(mt, _P)],
                            z1[:, kt, :],
                            start=(kt == 0),
                            stop=(kt == _HK - 1),
                        )
                    nc.scalar.activation(
                        fT[:, mt, :],
                        ps[:],
                        mybir.ActivationFunctionType.Relu,
                        bias=ba2_t[:, mt : mt + 1],
                    )
                return fT

            fpT = attend_T(ptb_t)
            nc.sync.dma_start(fpT_o[:].bitcast(F32R), fpT[:])
            fhT = attend_T(htb_t)
            nc.scalar.dma_start(fhT_o[:].bitcast(F32R), fhT[:])

            def transpose_16(src):
                rm = ob.tile([_P, _HK, _H], F32R, tag=f"rm{src is fhT}")
                for i in range(_HK):
                    for j in range(_HK):
                        tp = pp.tile([_P, _P], F32, tag="tps")
                        nc.tensor.transpose(
                            tp[:], src[:, i, ts(j, _P)].bitcast(F32), ident[:]
                        )
                        nc.vector.tensor_copy(rm[:, j, ts(i, _P)], tp[:].bitcast(F32R))
                return rm

            st_sb = ob.tile([_P, 2, _HK, _D], F32)
            # T_r[k, d] = sum_c fh_r[k, c] * h_blk[c, d]  (lhsT = fhT directly)
            for mt in range(_HK):
                ps = pp.tile([_P, _D], F32, tag="stps")
                for kt in range(_HK):
                    nc.tensor.matmul(
                        ps[:],
                        fhT[:, kt, ts(mt, _P)],
                        hblk_t[:, kt, :],
                        start=(kt == 0),
                        stop=(kt == _HK - 1),
                    )
                nc.vector.tensor_copy(st_sb[:, 1, mt, :], ps[:])
            nc.scalar.dma_start(ST_o[:, _HK * _D :], st_sb[:, 1, :, :])

            fh_rm = transpose_16(fhT)  # fh_r row-major: E diag chunk's rhs

            # E diagonal block: fp_c @ G[:, r*512:(r+1)*512] = fp_c @ fh_r
            ed = ob.tile([_P, _HK, _B], F32)
            for mt in range(_HK):
                ps = ep.tile([_P, _B], F32, tag="edps")
                for kt in range(_HK):
                    nc.tensor.matmul(
                        ps[:],
                        fpT[:, kt, ts(mt, _P)],
                        fh_rm[:, kt, :],
                        start=(kt == 0),
                        stop=(kt == _HK - 1),
                    )
                nc.vector.tensor_copy(ed[:, mt, :], ps[:])
            nc.scalar.dma_start(ED_o[:], ed[:])

            fp_rm = transpose_16(fpT)  # fp row-major: S's lhsT

            # S_c[k, d] = sum_i fp[i, k] * p_emb[i, d]
            for mt in range(_HK):
                ps = pp.tile([_P, _D], F32, tag="stps")
                for kt in range(_HK):
                    nc.tensor.matmul(
                        ps[:],
                        fp_rm[:, kt, ts(mt, _P)],
                        pblk_t[:, kt, :],
                        start=(kt == 0),
                        stop=(kt == _HK - 1),
                    )
                nc.vector.tensor_copy(st_sb[:, 0, mt, :], ps[:])
            nc.sync.dma_start(ST_o[:, : _HK * _D], st_sb[:, 0, :, :])

    nc.compile()
    return nc


def _build_l2():
    import concourse.bacc as bacc
    import concourse.bass as bass
    import concourse.mybir as mybir
    import concourse.tile as tile
    from concourse.masks import make_identity

    F32 = mybir.dt.float32
    F32R = mybir.dt.float32r
    F16 = mybir.dt.float16
    ts = bass.ts

    nc = bacc.Bacc("TRN2", target_bir_lowering=False, debug=False, num_devices=_NCORES)

    pk_a = nc.dram_tensor("pk_a", [_P, _PK2A.total], F16, kind="ExternalInput")
    pk_b = nc.dram_tensor("pk_b", [_P, _PK2B.total], F32, kind="ExternalInput")
    pk_c = nc.dram_tensor("pk_c", [_P, _PK2C.total], F16, kind="ExternalInput")
    # G packed chunk-major, 7 per-core-rotated chunks: [p][j][kt][512]
    G_i = nc.dram_tensor("G", [_P, _NE * _HK * _B], F16, kind="ExternalInput")

    # E packed chunk-major [p][j][mt][512]; host unswizzles + unrotates
    E_o = nc.dram_tensor("E", [_P, _NE * _HK * _B], F16, kind="ExternalOutput")
    ba_o = nc.dram_tensor("ba", [_P, 2 * _HK * _D], F32, kind="ExternalOutput")
    v_o = nc.dram_tensor("v", [_P, 2 * _HK], F32, kind="ExternalOutput")

    with tile.TileContext(nc) as tc:
        with (
            tc.tile_pool(name="consts", bufs=1) as cb,
            tc.tile_pool(name="one", bufs=1) as ob,
            tc.tile_pool(name="gstream", bufs=4) as gb,
            tc.tile_pool(name="sbuf", bufs=2) as sb,
            tc.tile_pool(name="esb", bufs=3) as eb,
            tc.tile_pool(name="psum", bufs=2, space="PSUM") as pp,
            tc.tile_pool(name="epsum", bufs=4, space="PSUM") as ep,
        ):
            ta = cb.tile([_P, _PK2A.total], F16)
            nc.sync.dma_start(ta[:], pk_a[:])
            tb = cb.tile([_P, _PK2B.total], F32R)
            tc16 = cb.tile([_P, _PK2C.total], F16)

            ident = cb.tile([_P, _P], F32)
            make_identity(nc, ident[:])

            warm_ps = pp.tile([_P, _B], F32, tag="compps")
            for _ in range(24):
                nc.tensor.matmul(
                    warm_ps[:, 0:_P], ident[:], ident[:], start=True, stop=True
                )

            fpT16 = _PK2A.view(ta, "fpT16")
            Tg = _PK2B.view(tb, "Tg")
            Ss = _PK2B.view(tb, "Ss")
            fpT = _PK2B.view(tb, "fpT")
            fhr = _PK2B.view(tb, "fhr")
            bc1 = _PK2B.view(tb, "bc1")[:, 0, :].bitcast(F32)
            bc2 = _PK2B.view(tb, "bc2")[:, 0, :].bitcast(F32)
            wc1p = _PK2C.view(tc16, "wc1p")
            wc1b = _PK2C.view(tc16, "wc1b")
            wc2 = _PK2C.view(tc16, "wc2")
            pT = _PK2C.view(tc16, "pT")
            hT = _PK2C.view(tc16, "hT")

            # ---- E row-block (7 rotated chunks), streamed; highest priority
            g_dmas = []
            for j in range(_NE):
                gt = gb.tile([_P, _HK, _B], F16, tag="gchunk")
                g_dmas.append(
                    nc.sync.dma_start(
                        gt[:],
                        G_i[:, j * _HK * _B : (j + 1) * _HK * _B].rearrange(
                            "p (t n) -> p t n", t=_HK
                        ),
                    )
                )
                es = eb.tile([_P, _HK, _B], F16, tag="esb")
                for mt in range(_HK):
                    ps = ep.tile([_P, _B], F32, tag="eps")
                    for kt in range(_HK):
                        nc.tensor.matmul(
                            ps[:],
                            fpT16[:, kt, ts(mt, _P)],
                            gt[:, kt, :],
                            start=(kt == 0),
                            stop=(kt == _HK - 1),
                        )
                    nc.vector.tensor_copy(es[:, mt, :], ps[:])
                nc.scalar.dma_start(
                    E_o[:, j * _HK * _B : (j + 1) * _HK * _B], es[:]
                )

            # beta/alpha/comp inputs: keep their transfers off the wire
            # until the first E chunks (the critical path) have landed
            from concourse.tile import add_dep_helper

            tb_dma = nc.scalar.dma_start(tb[:], pk_b[:].bitcast(F32R))
            add_dep_helper(tb_dma.ins, g_dmas[1].ins, sync=True,
                           reason="delay beta/alpha pack behind G stream")
            tc_dma = nc.scalar.dma_start(tc16[:], pk_c[:])
            add_dep_helper(tc_dma.ins, g_dmas[3].ins, sync=True,
                           reason="delay comp pack behind G stream")

            ba_sb = ob.tile([_P, 2, _HK, _D], F32)

            def normalized_block(lhsT_tile, rhs_tile, slot, tag):
                """row-major block + normalize by col 300; also emit the
                feature-major f32r transpose for comp()."""
                rec = ob.tile([_P, _HK], F32, tag=f"{tag}rec")
                rm = ob.tile([_P, _HK, _DPAD], F32, tag="normrm")
                nc.vector.memset(rm[:], 0.0)
                for mt in range(_HK):
                    ps = pp.tile([_P, _DN], F32, tag="normps")
                    for kt in range(_HK):
                        nc.tensor.matmul(
                            ps[:],
                            lhsT_tile[:, kt, ts(mt, _P)],
                            rhs_tile[:, kt, :],
                            start=(kt == 0),
                            stop=(kt == _HK - 1),
                        )
                    nc.vector.reciprocal(rec[:, mt : mt + 1], ps[:, _D : _D + 1])
                    nc.vector.tensor_scalar_mul(
                        rm[:, mt, 0:_D], ps[:, 0:_D], rec[:, mt : mt + 1]
                    )
                    nc.vector.tensor_copy(ba_sb[:, slot, mt, :], rm[:, mt, 0:_D])
                tT = ob.tile([_P, _DK, _B], F16, tag=tag)
                for i in range(_HK):
                    for j in range(_DK):
                        tp = pp.tile([_P, _P], F32, tag="compps")
                        nc.tensor.transpose(tp[:], rm[:, i, ts(j, _P)], ident[:])
                        nc.vector.tensor_copy(tT[:, j, ts(i, _P)], tp[:])
                return tT

            betaT = normalized_block(fpT, Tg, 0, "betaT")
            alphaT = normalized_block(fhr, Ss, 1, "alphaT")
            nc.scalar.dma_start(ba_o[:], ba_sb[:])

            v_sb = ob.tile([_P, 2, _HK], F32)

            def comp_partial(embT, xT, slot, tag):
                z1 = ob.tile([_P, _HK, _B], F16, tag="compz1")
                for mt in range(_HK):
                    ps = pp.tile([_P, _B], F32, tag="compps")
                    for kt in range(_DK):
                        nc.tensor.matmul(
                            ps[:],
                            wc1p[:, kt, ts(mt, _P)],
                            embT[:, kt, :],
                            start=(kt == 0),
                            stop=False,
                        )
                    for kt in range(_DK):
                        nc.tensor.matmul(
                            ps[:],
                            wc1b[:, kt, ts(mt, _P)],
                            xT[:, kt, :],
                            start=False,
                            stop=(kt == _DK - 1),
                        )
                    nc.scalar.activation(
                        z1[:, mt, :],
                        ps[:],
                        mybir.ActivationFunctionType.Relu,
                        bias=bc1[:, mt : mt + 1],
                    )
                for mt in range(_HK):
                    z2 = sb.tile([_P, _B], F32, tag=f"c{tag}z2")
                    ps = pp.tile([_P, _B], F32, tag="compps")
                    for kt in range(_HK):
                        nc.tensor.matmul(
                            ps[:],
                            wc2[:, kt, ts(mt, _P)],
                            z1[:, kt, :],
                            start=(kt == 0),
                            stop=(kt == _HK - 1),
                        )
                    nc.scalar.activation(
                        z2[:],
                        ps[:],
                        mybir.ActivationFunctionType.Relu,
                        bias=bc2[:, mt : mt + 1],
                    )
                    nc.vector.reduce_sum(
                        v_sb[:, slot, mt : mt + 1], z2[:], axis=mybir.AxisListType.X
                    )

            comp_partial(pT, betaT, 0, "1")
            comp_partial(hT, alphaT, 1, "2")
            nc.scalar.dma_start(v_o[:], v_sb[:])

    nc.compile()
    return nc


def _get(name):
    if name not in _cache:
        _cache[name] = _build_l1() if name == "l1" else _build_l2()
    return _cache[name]


def kernel(
    p_idx,
    h_idx,
    emb,
    W_a1,
    b_a1,
    W_a2,
    b_a2,
    W_c1,
    b_c1,
    W_c2,
    b_c2,
    W_g1,
    b_g1,
    W_g2,
    b_g2,
    W_g3,
    b_g3,
):
    from concourse.bass_utils import run_bass_kernel_spmd

    f32 = np.float32
    emb = np.asarray(emb, f32)
    cores = list(range(_NCORES))

    # ---- shard inputs: row-lookup + slice per core ----
    p_emb = np.ascontiguousarray(emb[np.asarray(p_idx, np.int64)])  # [4096, 300]
    h_emb = np.ascontiguousarray(emb[np.asarray(h_idx, np.int64)])

    ones = np.ones((1, _B), f32)
    w1b = _pad_rows(
        np.vstack([np.asarray(W_a1, f32).T, np.asarray(b_a1, f32)[None, :]]), _DPAD
    )
    w2 = np.asarray(W_a2, f32).T
    ba2 = np.asarray(b_a2, f32).reshape(_HK, _P).T  # [128, 4]

    in_maps1 = []
    for c in range(_NCORES):
        pb = p_emb[c * _B : (c + 1) * _B]
        hs = h_emb[c::_NCORES]
        in_maps1.append(
            {
                "pk_a": _PK1A.build(
                    {
                        "w1b": w1b,
                        "ptb": _pad_rows(np.vstack([pb.T, ones]), _DPAD),
                        "pblk": pb,
                        "hblk": h_emb[c * _B : (c + 1) * _B],
                    }
                ),
                "pk_b": _PK1B.build(
                    {
                        "w2": w2,
                        "ba2": ba2,
                        "htb": _pad_rows(np.vstack([hs.T, ones]), _DPAD),
                    }
                ),
            }
        )

    res1 = run_bass_kernel_spmd(_get("l1"), in_maps1, core_ids=cores)
    LAST_RESULTS.clear()
    LAST_RESULTS.append(res1)
    r1 = res1.results

    # ---- host glue: tiny sums + assembly ----
    fpT_blocks = [_unswz(r["fpT"], _HK) for r in r1]  # [512(feat), 512(row)]
    fhT_blocks = [_unswz(r["fhT"], _HK) for r in r1]
    ST = [_unswz(r["ST"], 2 * _HK) for r in r1]  # [8*128, 300] = [S; T]
    ED = [_unswz(r["ED"], _HK) for r in r1]  # [512, 512] diag E blocks
    fh = np.empty((_L, _H), f32)
    for r in range(_NCORES):
        fh[r::_NCORES] = fhT_blocks[r].T
    G = fh.reshape(_H, _L)
    # per-chunk swizzled views of G: chunk nn -> [128, HK*B]
    G_chunks = [
        _swz(np.ascontiguousarray(G[:, nn * _B : (nn + 1) * _B]), _HK).astype(
            np.float16
        )
        for nn in range(_NCORES)
    ]
    S = np.sum([st[:_H] for st in ST], axis=0, dtype=f32)
    T = np.sum([st[_H:] for st in ST], axis=0, dtype=f32)
    sfp = np.sum([b.sum(axis=1, dtype=np.float64) for b in fpT_blocks], axis=0)
    g = G.sum(axis=1, dtype=np.float64)
    zc = np.zeros((_H, 1), f32)
    Ss = np.hstack([S, sfp[:, None].astype(f32), zc])
    Tg = np.hstack([T, g[:, None].astype(f32), zc])

    wc1p = _pad_rows(np.asarray(W_c1, f32)[:, :_D].T, _DPAD)
    wc1b = _pad_rows(np.asarray(W_c1, f32)[:, _D:].T, _DPAD)
    bc1 = np.asarray(b_c1, f32).reshape(_HK, _P).T
    wc2 = np.asarray(W_c2, f32).T
    bc2 = np.asarray(b_c2, f32).reshape(_HK, _P).T

    in_maps2 = []
    for c in range(_NCORES):
        pb = p_emb[c * _B : (c + 1) * _B]
        hb = h_emb[c * _B : (c + 1) * _B]
        perm = [(c + 1 + j) % _NCORES for j in range(_NE)]
        in_maps2.append(
            {
                "pk_a": _PK2A.build({"fpT16": fpT_blocks[c]}, np.float16),
                "pk_b": _PK2B.build(
                    {
                        "Tg": Tg,
                        "Ss": Ss,
                        "fpT": fpT_blocks[c],
                        "fhr": fhT_blocks[c].T,
                        "bc1": bc1,
                        "bc2": bc2,
                    }
                ),
                "pk_c": _PK2C.build(
                    {
                        "wc1p": wc1p,
                        "wc1b": wc1b,
                        "wc2": wc2,
                        "pT": _pad_rows(pb.T, _DPAD),
                        "hT": _pad_rows(hb.T, _DPAD),
                    },
                    np.float16,
                ),
                "G": np.ascontiguousarray(
                    np.concatenate([G_chunks[nn] for nn in perm], axis=1)
                ),
            }
        )

    res2 = run_bass_kernel_spmd(_get("l2"), in_maps2, core_ids=cores)
    LAST_RESULTS.append(res2)
    r2 = res2.results

    # ---- gather/unshard ----
    E = np.empty((_L, _L), f32)
    for c in range(_NCORES):
        rows = slice(c * _B, (c + 1) * _B)
        E[rows, c * _B : (c + 1) * _B] = ED[c]
        eflat = r2[c]["E"].astype(f32)  # [128, 7*HK*B] chunk-major
        for j in range(_NE):
            nn = (c + 1 + j) % _NCORES
            E[rows, nn * _B : (nn + 1) * _B] = _unswz(
                eflat[:, j * _HK * _B : (j + 1) * _HK * _B], _HK
            )
    ba = [_unswz(r["ba"], 2 * _HK) for r in r2]  # [8*128, 300] = [beta; alpha]
    beta = np.concatenate([b[:_H] for b in ba], axis=0)
    alpha = np.concatenate([b[_H:] for b in ba], axis=0)
    v = np.sum([r["v"] for r in r2], axis=0, dtype=f32)  # [128, 2*HK]
    v1 = v[:, :_HK].T.reshape(_H)
    v2 = v[:, _HK:].T.reshape(_H)

    # final head: [1024] -> 512 -> 512 -> 3 (tiny; host fp32)
    y = np.concatenate([v1, v2])
    y = np.maximum(y @ np.asarray(W_g1, f32).T + np.asarray(b_g1, f32), 0.0)
    y = np.maximum(y @ np.asarray(W_g2, f32).T + np.asarray(b_g2, f32), 0.0)
    y = y @ np.asarray(W_g3, f32).T + np.asarray(b_g3, f32)
    y = y - y.max()
    ey = np.exp(y)
    y = (ey / ey.sum()).astype(f32)

    return (E, beta, alpha, v1, v2, y)


# revision 28
# speedup vs baseline: 1.0534x; 1.0124x over previous
"""Trainium2 Bass kernel for nn_Discriminator (decomposable attention over
gathered embeddings).

Math (reference):
    p_emb = emb[p_idx]; h_emb = emb[h_idx]                # [4096, 300]
    fp = attend(p_emb); fh = attend(h_emb)                # [4096, 512]
    G  = fh.reshape(512, 4096)      (row-major reshape)
    E  = fp @ G                                           # [4096, 4096]
    eik = E.sum(1); ekj = E.sum(0)
    beta  = (E/eik) @ h_emb;  alpha = (E/ekj).T @ p_emb   # [4096, 300]
    v1 = comp([p_emb|beta]).sum(0); v2 = comp([h_emb|alpha]).sum(0)
    y  = softmax(mlp([v1|v2]))                            # [3]

Key identities used to shard across 8 cores without collectives
(G[k, r*512+c] == fh[8k+r, c], so G's column block r is fh[r::8]):
    eik = fp @ g,          g = G.sum(1)
    E @ h_emb = fp @ T,    T = sum_r fh[r::8] @ h_emb[r*512:(r+1)*512]
    ekj[r*512+c] = (fh[r::8].T @ sfp)[c],   sfp = fp.sum(0)
    (E.T @ p_emb)[r*512:(r+1)*512] = fh[r::8].T @ S,   S = fp.T @ p_emb

Two SPMD launches on cores 0-7:
    L1: per-core attend on its p-block (rows c*512:(c+1)*512) and its strided
        h-slice (rows r::8); partial S_c, T_r; the diagonal E block
        fp_c @ fh_r (core c owns both operands).  Host sums S/T (tiny) and
        assembles G.
    L2: per-core remaining 7 E column chunks of its row block (G pack is
        rotated per core so the single NEFF stays SPMD), beta/alpha blocks,
        v1/v2 partials.
Host does only O(KB) glue plus the final 3-way MLP head on [v1|v2].

All device inputs/outputs are pre-swizzled on the host into [128, N]
partition-major flats so every DMA is one contiguous line per partition
(sequencer descriptor-generation cost was a profiled bottleneck), and input
packs are ordered so first-needed operands complete first (DMA bandwidth is
the other profiled bottleneck).
"""

import numpy as np

_P = 128
_D = 300
_H = 512
_L = 4096
_B = 512  # rows per core
_NCORES = 8
_DPAD = 384  # 300 padded up to 3*128 (row 300 carries the ones/bias trick)
_DN = _D + 2  # fp32r needs an even moving dim; col 300 = normalizer, 301 pad

_HK = _H // _P  # 4
_DK = _DPAD // _P  # 3
_NE = _NCORES - 1  # 7 off-diagonal E column chunks in L2

_cache = {}
LAST_RESULTS = []  # BassKernelResults of the most recent kernel() launches


def _swz(a, t):
    """[t*128, n] row-major -> [128, t*n] partition-major flat."""
    n = a.shape[1]
    return a.reshape(t, _P, n).transpose(1, 0, 2).reshape(_P, t * n)


def _unswz(a, t):
    """[128, t*n] partition-major flat -> [t*128, n] row-major."""
    n = a.shape[1] // t
    return a.reshape(_P, t, n).transpose(1, 0, 2).reshape(t * _P, n)


def _pad_rows(a, rows):
    out = np.zeros((rows, a.shape[1]), np.float32)
    out[: a.shape[0]] = a
    return out


class _Pack:
    """Host-side [128, N] pack builder + device-side view registry."""

    def __init__(self):
        self.specs = []  # (name, off, t, n)
        self.total = 0

    def add(self, name, t, n):
        self.specs.append((name, self.total, t, n))
        self.total += t * n

    def off(self, name):
        for nm, off, t, n in self.specs:
            if nm == name:
                return off, t, n
        raise KeyError(name)

    def view(self, tile, name):
        off, t, n = self.off(name)
        return tile[:, off : off + t * n].rearrange("p (t n) -> p t n", t=t)

    def range_of(self, names):
        offs = []
        for nm in names:
            off, t, n = self.off(nm)
            offs.append((off, off + t * n))
        lo = min(o for o, _ in offs)
        hi = max(e for _, e in offs)
        assert hi - lo == sum(e - o for o, e in offs), "group must be contiguous"
        return lo, hi

    def build(self, arrays, dtype=np.float32):
        """arrays: {name: [t*128, n] array}; returns [128, total] in dtype."""
        out = np.empty((_P, self.total), dtype)
        for nm, off, t, n in self.specs:
            a = arrays[nm]
            assert a.shape == (t * _P, n), (nm, a.shape, (t * _P, n))
            out[:, off : off + t * n] = _swz(
                np.ascontiguousarray(a).astype(dtype), t
            )
        return out


# ---- pack layouts (module-level so host and builder agree) ----
_PK1A = _Pack()  # L1 sync: attend-p operands, then S/T rhs
_PK1A.add("w1b", _DK, _H)
_PK1A.add("ptb", _DK, _B)
_PK1A.add("pblk", _HK, _D)
_PK1A.add("hblk", _HK, _D)
_PK1B = _Pack()  # L1 scalar: attend layer-2 weights + attend-h input
_PK1B.add("w2", _HK, _H)
_PK1B.add("ba2", 1, _HK)
_PK1B.add("htb", _DK, _B)

_PK2A = _Pack()  # L2 sync fp16, ahead of the G chunks: E lhsT
_PK2A.add("fpT16", _HK, _B)
_PK2B = _Pack()  # L2 scalar f32: beta/alpha operands (f32r quality path)
_PK2B.add("Tg", _HK, _DN)
_PK2B.add("Ss", _HK, _DN)
_PK2B.add("fpT", _HK, _B)
_PK2B.add("fhr", _HK, _H)
_PK2B.add("bc1", 1, _HK)
_PK2B.add("bc2", 1, _HK)
_PK2C = _Pack()  # L2 scalar fp16: comp operands
_PK2C.add("wc1p", _DK, _H)
_PK2C.add("wc1b", _DK, _H)
_PK2C.add("wc2", _HK, _H)
_PK2C.add("pT", _DK, _B)
_PK2C.add("hT", _DK, _B)


def _build_l1():
    import concourse.bacc as bacc
    import concourse.bass as bass
    import concourse.mybir as mybir
    import concourse.tile as tile
    from concourse.masks import make_identity

    F32 = mybir.dt.float32
    F32R = mybir.dt.float32r
    ts = bass.ts

    nc = bacc.Bacc("TRN2", target_bir_lowering=False, debug=False, num_devices=_NCORES)

    pk_a = nc.dram_tensor("pk_a", [_P, _PK1A.total], F32, kind="ExternalInput")
    pk_b = nc.dram_tensor("pk_b", [_P, _PK1B.total], F32, kind="ExternalInput")

    fpT_o = nc.dram_tensor("fpT", [_P, _HK * _B], F32, kind="ExternalOutput")
    fhT_o = nc.dram_tensor("fhT", [_P, _HK * _B], F32, kind="ExternalOutput")
    ST_o = nc.dram_tensor("ST", [_P, 2 * _HK * _D], F32, kind="ExternalOutput")
    ED_o = nc.dram_tensor("ED", [_P, _HK * _B], F32, kind="ExternalOutput")

    with tile.TileContext(nc) as tc:
        with (
            tc.tile_pool(name="consts", bufs=1) as cb,
            tc.tile_pool(name="one", bufs=1) as ob,
            tc.tile_pool(name="sbuf", bufs=2) as sb,
            tc.tile_pool(name="psum", bufs=2, space="PSUM") as pp,
            tc.tile_pool(name="edpsum", bufs=2, space="PSUM") as ep,
        ):
            ta = cb.tile([_P, _PK1A.total], F32R)
            lo, hi = _PK1A.range_of(["w1b", "ptb"])
            nc.sync.dma_start(ta[:, lo:hi], pk_a[:, lo:hi].bitcast(F32R))
            lo2, hi2 = _PK1A.range_of(["pblk", "hblk"])
            nc.sync.dma_start(ta[:, lo2:hi2], pk_a[:, lo2:hi2].bitcast(F32R))
            tb = cb.tile([_P, _PK1B.total], F32R)
            nc.scalar.dma_start(tb[:], pk_b[:].bitcast(F32R))

            ident = cb.tile([_P, _P], F32)
            make_identity(nc, ident[:])

            # PE pre-warm: ~4us of junk matmuls during the input-DMA wait so
            # the HAM clock gate opens (1.2 -> 2.4 GHz) before real work.
            warm_ps = pp.tile([_P, _B], F32, tag="attps")
            for _ in range(24):
                nc.tensor.matmul(
                    warm_ps[:, 0:_P], ident[:], ident[:], start=True, stop=True
                )

            w1b_t = _PK1A.view(ta, "w1b")
            ptb_t = _PK1A.view(ta, "ptb")
            pblk_t = _PK1A.view(ta, "pblk")
            hblk_t = _PK1A.view(ta, "hblk")
            w2_t = _PK1B.view(tb, "w2")
            ba2_t = _PK1B.view(tb, "ba2")[:, 0, :].bitcast(F32)
            htb_t = _PK1B.view(tb, "htb")

            def attend_T(xt):
                z1 = sb.tile([_P, _HK, _B], F32R, tag="attz1")
                for mt in range(_HK):
                    ps = pp.tile([_P, _B], F32, tag="attps")
                    for kt in range(_DK):
                        nc.tensor.matmul(
                            ps[:],
                            w1b_t[:, kt, ts(mt, _P)],
                            xt[:, kt, :],
                            start=(kt == 0),
                            stop=(kt == _DK - 1),
                        )
                    nc.scalar.activation(
                        z1[:, mt, :], ps[:], mybir.ActivationFunctionType.Relu
                    )
                fT = sb.tile([_P, _HK, _B], F32R, tag="attout")
                for mt in range(_HK):
                    ps = pp.tile([_P, _B], F32, tag="attps")
                    for kt in range(_HK):
                        nc.tensor.matmul(
                            ps[:],
                            w2_t[:, kt, ts(mt, _P)],
                            z1[:, kt, :],
                            start=(kt == 0),
                            stop=(kt == _HK - 1),
                        )
                    nc.scalar.activation(
                        fT[:, mt, :],
                        ps[:],
                        mybir.ActivationFunctionType.Relu,
                        bias=ba2_t[:, mt : mt + 1],
                    )
                return fT

            fpT = attend_T(ptb_t)
            nc.sync.dma_start(fpT_o[:].bitcast(F32R), fpT[:])
            fhT = attend_T(htb_t)
            nc.scalar.dma_start(fhT_o[:].bitcast(F32R), fhT[:])

            def transpose_16(src):
                rm = ob.tile([_P, _HK, _H], F32R, tag=f"rm{src is fhT}")
                for i in range(_HK):
                    for j in range(_HK):
                        tp = pp.tile([_P, _P], F32, tag="tps")
                        nc.tensor.transpose(
                            tp[:], src[:, i, ts(j, _P)].bitcast(F32), ident[:]
                        )
                        nc.vector.tensor_copy(rm[:, j, ts(i, _P)], tp[:].bitcast(F32R))
                return rm

            st_sb = ob.tile([_P, 2, _HK, _D], F32)
            # T_r[k, d] = sum_c fh_r[k, c] * h_blk[c, d]  (lhsT = fhT directly)
            for mt in range(_HK):
                ps = pp.tile([_P, _D], F32, tag="stps")
                for kt in range(_HK):
                    nc.tensor.matmul(
                        ps[:],
                        fhT[:, kt, ts(mt, _P)],
                        hblk_t[:, kt, :],
                        start=(kt == 0),
                        stop=(kt == _HK - 1),
                    )
                nc.vector.tensor_copy(st_sb[:, 1, mt, :], ps[:])
            nc.scalar.dma_start(ST_o[:, _HK * _D :], st_sb[:, 1, :, :])

            fh_rm = transpose_16(fhT)  # fh_r row-major: E diag chunk's rhs

            # E diagonal block: fp_c @ G[:, r*512:(r+1)*512] = fp_c @ fh_r
            ed = ob.tile([_P, _HK, _B], F32)
            for mt in range(_HK):
                ps = ep.tile([_P, _B], F32, tag="edps")
                for kt in range(_HK):
                    nc.tensor.matmul(
                        ps[:],
                        fpT[:, kt, ts(mt, _P)],
                        fh_rm[:, kt, :],
                        start=(kt == 0),
                        stop=(kt == _HK - 1),
                    )
                nc.vector.tensor_copy(ed[:, mt, :], ps[:])
            nc.scalar.dma_start(ED_o[:], ed[:])

            fp_rm = transpose_16(fpT)  # fp row-major: S's lhsT

            # S_c[k, d] = sum_i fp[i, k] * p_emb[i, d]
            for mt in range(_HK):
                ps = pp.tile([_P, _D], F32, tag="stps")
                for kt in range(_HK):
                    nc.tensor.matmul(
                        ps[:],
                        fp_rm[:, kt, ts(mt, _P)],
                        pblk_t[:, kt, :],
                        start=(kt == 0),
                        stop=(kt == _HK - 1),
                    )
                nc.vector.tensor_copy(st_sb[:, 0, mt, :], ps[:])
            nc.sync.dma_start(ST_o[:, : _HK * _D], st_sb[:, 0, :, :])

    nc.compile()
    return nc


def _build_l2():
    import concourse.bacc as bacc
    import concourse.bass as bass
    import concourse.mybir as mybir
    import concourse.tile as tile
    from concourse.masks import make_identity

    F32 = mybir.dt.float32
    F32R = mybir.dt.float32r
    F16 = mybir.dt.float16
    ts = bass.ts

    nc = bacc.Bacc("TRN2", target_bir_lowering=False, debug=False, num_devices=_NCORES)

    pk_a = nc.dram_tensor("pk_a", [_P, _PK2A.total], F16, kind="ExternalInput")
    pk_b = nc.dram_tensor("pk_b", [_P, _PK2B.total], F32, kind="ExternalInput")
    pk_c = nc.dram_tensor("pk_c", [_P, _PK2C.total], F16, kind="ExternalInput")
    # G packed chunk-major, 7 per-core-rotated chunks: [p][j][kt][512]
    G_i = nc.dram_tensor("G", [_P, _NE * _HK * _B], F16, kind="ExternalInput")

    # E packed chunk-major [p][j][mt][512]; host unswizzles + unrotates
    E_o = nc.dram_tensor("E", [_P, _NE * _HK * _B], F16, kind="ExternalOutput")
    ba_o = nc.dram_tensor("ba", [_P, 2 * _HK * _D], F32, kind="ExternalOutput")
    v_o = nc.dram_tensor("v", [_P, 2 * _HK], F32, kind="ExternalOutput")

    with tile.TileContext(nc) as tc:
        with (
            tc.tile_pool(name="consts", bufs=1) as cb,
            tc.tile_pool(name="one", bufs=1) as ob,
            tc.tile_pool(name="gstream", bufs=4) as gb,
            tc.tile_pool(name="sbuf", bufs=2) as sb,
            tc.tile_pool(name="esb", bufs=3) as eb,
            tc.tile_pool(name="psum", bufs=2, space="PSUM") as pp,
            tc.tile_pool(name="epsum", bufs=4, space="PSUM") as ep,
        ):
            ta = cb.tile([_P, _PK2A.total], F16)
            nc.sync.dma_start(ta[:], pk_a[:])
            tb = cb.tile([_P, _PK2B.total], F32R)
            tc16 = cb.tile([_P, _PK2C.total], F16)

            ident = cb.tile([_P, _P], F32)
            make_identity(nc, ident[:])

            warm_ps = pp.tile([_P, _B], F32, tag="compps")
            for _ in range(24):
                nc.tensor.matmul(
                    warm_ps[:, 0:_P], ident[:], ident[:], start=True, stop=True
                )

            fpT16 = _PK2A.view(ta, "fpT16")
            Tg = _PK2B.view(tb, "Tg")
            Ss = _PK2B.view(tb, "Ss")
            fpT = _PK2B.view(tb, "fpT")
            fhr = _PK2B.view(tb, "fhr")
            bc1 = _PK2B.view(tb, "bc1")[:, 0, :].bitcast(F32)
            bc2 = _PK2B.view(tb, "bc2")[:, 0, :].bitcast(F32)
            wc1p = _PK2C.view(tc16, "wc1p")
            wc1b = _PK2C.view(tc16, "wc1b")
            wc2 = _PK2C.view(tc16, "wc2")
            pT = _PK2C.view(tc16, "pT")
            hT = _PK2C.view(tc16, "hT")

            # ---- E row-block (7 rotated chunks), streamed; highest priority
            g_dmas = []
            for j in range(_NE):
                gt = gb.tile([_P, _HK, _B], F16, tag="gchunk")
                g_dmas.append(
                    nc.sync.dma_start(
                        gt[:],
                        G_i[:, j * _HK * _B : (j + 1) * _HK * _B].rearrange(
                            "p (t n) -> p t n", t=_HK
                        ),
                    )
                )
                es = eb.tile([_P, _HK, _B], F16, tag="esb")
                for mt in range(_HK):
                    ps = ep.tile([_P, _B], F32, tag="eps")
                    for kt in range(_HK):
                        nc.tensor.matmul(
                            ps[:],
                            fpT16[:, kt, ts(mt, _P)],
                            gt[:, kt, :],
                            start=(kt == 0),
                            stop=(kt == _HK - 1),
                        )
                    nc.vector.tensor_copy(es[:, mt, :], ps[:])
                nc.scalar.dma_start(
                    E_o[:, j * _HK * _B : (j + 1) * _HK * _B], es[:]
                )

            # beta/alpha/comp inputs: keep their transfers off the wire
            # until the first E chunks (the critical path) have landed
            from concourse.tile import add_dep_helper

            tb_dma = nc.scalar.dma_start(tb[:], pk_b[:].bitcast(F32R))
            add_dep_helper(tb_dma.ins, g_dmas[4].ins, sync=True,
                           reason="delay beta/alpha pack behind G stream")
            tc_dma = nc.scalar.dma_start(tc16[:], pk_c[:])
            add_dep_helper(tc_dma.ins, g_dmas[6].ins, sync=True,
                           reason="delay comp pack behind G stream")

            ba_sb = ob.tile([_P, 2, _HK, _D], F32)

            def normalized_block(lhsT_tile, rhs_tile, slot, tag):
                """row-major block + normalize by col 300; also emit the
                feature-major f32r transpose for comp()."""
                rec = ob.tile([_P, _HK], F32, tag=f"{tag}rec")
                rm = ob.tile([_P, _HK, _DPAD], F32, tag="normrm")
                nc.vector.memset(rm[:], 0.0)
                for mt in range(_HK):
                    ps = pp.tile([_P, _DN], F32, tag="normps")
                    for kt in range(_HK):
                        nc.tensor.matmul(
                            ps[:],
                            lhsT_tile[:, kt, ts(mt, _P)],
                            rhs_tile[:, kt, :],
                            start=(kt == 0),
                            stop=(kt == _HK - 1),
                        )
                    nc.vector.reciprocal(rec[:, mt : mt + 1], ps[:, _D : _D + 1])
                    nc.vector.tensor_scalar_mul(
                        rm[:, mt, 0:_D], ps[:, 0:_D], rec[:, mt : mt + 1]
                    )
                    nc.vector.tensor_copy(ba_sb[:, slot, mt, :], rm[:, mt, 0:_D])
                tT = ob.tile([_P, _DK, _B], F16, tag=tag)
                for i in range(_HK):
                    for j in range(_DK):
                        tp = pp.tile([_P, _P], F32, tag="compps")
                        nc.tensor.transpose(tp[:], rm[:, i, ts(j, _P)], ident[:])
                        nc.vector.tensor_copy(tT[:, j, ts(i, _P)], tp[:])
                return tT

            betaT = normalized_block(fpT, Tg, 0, "betaT")
            alphaT = normalized_block(fhr, Ss, 1, "alphaT")
            nc.scalar.dma_start(ba_o[:], ba_sb[:])

            v_sb = ob.tile([_P, 2, _HK], F32)

            def comp_partial(embT, xT, slot, tag):
                z1 = ob.tile([_P, _HK, _B], F16, tag="compz1")
                for mt in range(_HK):
                    ps = pp.tile([_P, _B], F32, tag="compps")
                    for kt in range(_DK):
                        nc.tensor.matmul(
                            ps[:],
                            wc1p[:, kt, ts(mt, _P)],
                            embT[:, kt, :],
                            start=(kt == 0),
                            stop=False,
                        )
                    for kt in range(_DK):
                        nc.tensor.matmul(
                            ps[:],
                            wc1b[:, kt, ts(mt, _P)],
                            xT[:, kt, :],
                            start=False,
                            stop=(kt == _DK - 1),
                        )
                    nc.scalar.activation(
                        z1[:, mt, :],
                        ps[:],
                        mybir.ActivationFunctionType.Relu,
                        bias=bc1[:, mt : mt + 1],
                    )
                for mt in range(_HK):
                    z2 = sb.tile([_P, _B], F32, tag=f"c{tag}z2")
                    ps = pp.tile([_P, _B], F32, tag="compps")
                    for kt in range(_HK):
                        nc.tensor.matmul(
                            ps[:],
                            wc2[:, kt, ts(mt, _P)],
                            z1[:, kt, :],
                            start=(kt == 0),
                            stop=(kt == _HK - 1),
                        )
                    nc.scalar.activation(
                        z2[:],
                        ps[:],
                        mybir.ActivationFunctionType.Relu,
                        bias=bc2[:, mt : mt + 1],
                    )
                    nc.vector.reduce_sum(
                        v_sb[:, slot, mt : mt + 1], z2[:], axis=mybir.AxisListType.X
                    )

            comp_partial(pT, betaT, 0, "1")
            comp_partial(hT, alphaT, 1, "2")
            nc.scalar.dma_start(v_o[:], v_sb[:])

    nc.compile()
    return nc


def _get(name):
    if name not in _cache:
        _cache[name] = _build_l1() if name == "l1" else _build_l2()
    return _cache[name]


def kernel(
    p_idx,
    h_idx,
    emb,
    W_a1,
    b_a1,
    W_a2,
    b_a2,
    W_c1,
    b_c1,
    W_c2,
    b_c2,
    W_g1,
    b_g1,
    W_g2,
    b_g2,
    W_g3,
    b_g3,
):
    from concourse.bass_utils import run_bass_kernel_spmd

    f32 = np.float32
    emb = np.asarray(emb, f32)
    cores = list(range(_NCORES))

    # ---- shard inputs: row-lookup + slice per core ----
    p_emb = np.ascontiguousarray(emb[np.asarray(p_idx, np.int64)])  # [4096, 300]
    h_emb = np.ascontiguousarray(emb[np.asarray(h_idx, np.int64)])

    ones = np.ones((1, _B), f32)
    w1b = _pad_rows(
        np.vstack([np.asarray(W_a1, f32).T, np.asarray(b_a1, f32)[None, :]]), _DPAD
    )
    w2 = np.asarray(W_a2, f32).T
    ba2 = np.asarray(b_a2, f32).reshape(_HK, _P).T  # [128, 4]

    in_maps1 = []
    for c in range(_NCORES):
        pb = p_emb[c * _B : (c + 1) * _B]
        hs = h_emb[c::_NCORES]
        in_maps1.append(
            {
                "pk_a": _PK1A.build(
                    {
                        "w1b": w1b,
                        "ptb": _pad_rows(np.vstack([pb.T, ones]), _DPAD),
                        "pblk": pb,
                        "hblk": h_emb[c * _B : (c + 1) * _B],
                    }
                ),
                "pk_b": _PK1B.build(
                    {
                        "w2": w2,
                        "ba2": ba2,
                        "htb": _pad_rows(np.vstack([hs.T, ones]), _DPAD),
                    }
                ),
            }
        )

    res1 = run_bass_kernel_spmd(_get("l1"), in_maps1, core_ids=cores)
    LAST_RESULTS.clear()
    LAST_RESULTS.append(res1)
    r1 = res1.results

    # ---- host glue: tiny sums + assembly ----
    fpT_blocks = [_unswz(r["fpT"], _HK) for r in r1]  # [512(feat), 512(row)]
    fhT_blocks = [_unswz(r["fhT"], _HK) for r in r1]
    ST = [_unswz(r["ST"], 2 * _HK) for r in r1]  # [8*128, 300] = [S; T]
    ED = [_unswz(r["ED"], _HK) for r in r1]  # [512, 512] diag E blocks
    fh = np.empty((_L, _H), f32)
    for r in range(_NCORES):
        fh[r::_NCORES] = fhT_blocks[r].T
    G = fh.reshape(_H, _L)
    # per-chunk swizzled views of G: chunk nn -> [128, HK*B]
    G_chunks = [
        _swz(np.ascontiguousarray(G[:, nn * _B : (nn + 1) * _B]), _HK).astype(
            np.float16
        )
        for nn in range(_NCORES)
    ]
    S = np.sum([st[:_H] for st in ST], axis=0, dtype=f32)
    T = np.sum([st[_H:] for st in ST], axis=0, dtype=f32)
    sfp = np.sum([b.sum(axis=1, dtype=np.float64) for b in fpT_blocks], axis=0)
    g = G.sum(axis=1, dtype=np.float64)
    zc = np.zeros((_H, 1), f32)
    Ss = np.hstack([S, sfp[:, None].astype(f32), zc])
    Tg = np.hstack([T, g[:, None].astype(f32), zc])

    wc1p = _pad_rows(np.asarray(W_c1, f32)[:, :_D].T, _DPAD)
    wc1b = _pad_rows(np.asarray(W_c1, f32)[:, _D:].T, _DPAD)
    bc1 = np.asarray(b_c1, f32).reshape(_HK, _P).T
    wc2 = np.asarray(W_c2, f32).T
    bc2 = np.asarray(b_c2, f32).reshape(_HK, _P).T

    in_maps2 = []
    for c in range(_NCORES):
        pb = p_emb[c * _B : (c + 1) * _B]
        hb = h_emb[c * _B : (c + 1) * _B]
        perm = [(c + 1 + j) % _NCORES for j in range(_NE)]
        in_maps2.append(
            {
                "pk_a": _PK2A.build({"fpT16": fpT_blocks[c]}, np.float16),
                "pk_b": _PK2B.build(
                    {
                        "Tg": Tg,
                        "Ss": Ss,
                        "fpT": fpT_blocks[c],
                        "fhr": fhT_blocks[c].T,
                        "bc1": bc1,
                        "bc2": bc2,
                    }
                ),
                "pk_c": _PK2C.build(
                    {
                        "wc1p": wc1p,
                        "wc1b": wc1b,
                        "wc2": wc2,
                        "pT": _pad_rows(pb.T, _DPAD),
                        "hT": _pad_rows(hb.T, _DPAD),
                    },
                    np.float16,
                ),
                "G": np.ascontiguousarray(
                    np.concatenate([G_chunks[nn] for nn in perm], axis=1)
                ),
            }
        )

    res2 = run_bass_kernel_spmd(_get("l2"), in_maps2, core_ids=cores)
    LAST_RESULTS.append(res2)
    r2 = res2.results

    # ---- gather/unshard ----
    E = np.empty((_L, _L), f32)
    for c in range(_NCORES):
        rows = slice(c * _B, (c + 1) * _B)
        E[rows, c * _B : (c + 1) * _B] = ED[c]
        eflat = r2[c]["E"].astype(f32)  # [128, 7*HK*B] chunk-major
        for j in range(_NE):
            nn = (c + 1 + j) % _NCORES
            E[rows, nn * _B : (nn + 1) * _B] = _unswz(
                eflat[:, j * _HK * _B : (j + 1) * _HK * _B], _HK
            )
    ba = [_unswz(r["ba"], 2 * _HK) for r in r2]  # [8*128, 300] = [beta; alpha]
    beta = np.concatenate([b[:_H] for b in ba], axis=0)
    alpha = np.concatenate([b[_H:] for b in ba], axis=0)
    v = np.sum([r["v"] for r in r2], axis=0, dtype=f32)  # [128, 2*HK]
    v1 = v[:, :_HK].T.reshape(_H)
    v2 = v[:, _HK:].T.reshape(_H)

    # final head: [1024] -> 512 -> 512 -> 3 (tiny; host fp32)
    y = np.concatenate([v1, v2])
    y = np.maximum(y @ np.asarray(W_g1, f32).T + np.asarray(b_g1, f32), 0.0)
    y = np.maximum(y @ np.asarray(W_g2, f32).T + np.asarray(b_g2, f32), 0.0)
    y = y @ np.asarray(W_g3, f32).T + np.asarray(b_g3, f32)
    y = y - y.max()
    ey = np.exp(y)
    y = (ey / ey.sum()).astype(f32)

    return (E, beta, alpha, v1, v2, y)
